# revision 32
# baseline (speedup 1.0000x reference)
"""Trainium2 Bass kernel for nn_BKG_encoder (sparse graph-transformer encoder).

Strategy:
- Pure data parallelism: 16 of the 128 independent 64-node subgraphs per core.
- Sparse attention (fixed 16 out-edges per node, edges stay inside each
  64-node graph) is recomputed as dense 64x64 masked attention, where the
  mask is the edge multiplicity matrix (built host-side from row/col).
  exp(S - C) * mult, row-normalized, is mathematically identical to the
  reference's edge softmax.
- Activations are stored channel-on-partition ("transposed", hT = (256,
  1024-local-nodes)) so projections/FFN/BN are natural; attention S^T is
  computed per head with 64x64 graph blocks packed 2-per-128-partitions.
- BatchNorm is exact: per-core sums go through a tiny (128,4) AllReduce.
- fp16 is used only inside attention (q/k/v/exp/attn); everything dense
  runs fp32r on the PE (full speed at free-dim >= 256) with fp32 storage.
"""
import os
import sys

for _p in ("/opt/trn_rl_repo", "/opt/pypackages"):
    if _p not in sys.path:
        sys.path.insert(0, _p)

import numpy as np

import concourse.bass as bass
import concourse.bacc as bacc
import concourse.tile as tile
from concourse import mybir
from concourse.masks import make_identity
from concourse.bass_utils import run_bass_kernel_spmd

F32 = mybir.dt.float32
F32R = mybir.dt.float32r
F16 = mybir.dt.float16

HID = 256
HEADS = 8
HD = HID // HEADS          # 32
LAYERS = 3
N = 8192
B = 128
NPER = N // B              # 64
DEG = 16
EPS = 1e-5
SCALE = HD ** -0.5

NCORES = 8
BC = B // NCORES           # 16 graphs per core
NL = N // NCORES           # 1024 nodes per core
PAIRS = NL // 128          # 8 pairs of graphs (128-node blocks)
EXPC = 6.0                 # constant subtracted inside exp for fp16 range

# head-major channel permutation: PERM[32h + d] = d*HEADS + h
PERM = np.array([d * HEADS + h for h in range(HEADS) for d in range(HD)], np.int64)

AL = mybir.AluOpType


def r32(ap):
    return ap.bitcast(F32R)


def _ap_append(ap, stride, size):
    """Append an innermost free dim [stride, size] to an AP."""
    return bass.AP(tensor=ap.tensor, offset=ap.offset, ap=[*ap.ap, [stride, size]])


def build_program():
    nc = bacc.Bacc("TRN2", target_bir_lowering=False, debug=False,
                   num_devices=NCORES)

    # ---- DRAM parameters (per-core inputs) ----
    d_hT = nc.declare_dram_parameter("hT", [HID, NL], F32, isOutput=False)
    d_mask = nc.declare_dram_parameter("maskT", [128, 512], F16, isOutput=False)
    d_wq = nc.declare_dram_parameter("wq", [LAYERS, HID, HID], F16, isOutput=False)
    d_wk = nc.declare_dram_parameter("wk", [LAYERS, HID, HID], F16, isOutput=False)
    d_wv = nc.declare_dram_parameter("wv", [LAYERS, HID, HID], F16, isOutput=False)
    d_wo = nc.declare_dram_parameter("wo", [LAYERS, HID, HID], F16, isOutput=False)
    d_w1 = nc.declare_dram_parameter("w1", [LAYERS, HID, 2 * HID], F16, isOutput=False)
    d_w2 = nc.declare_dram_parameter("w2", [LAYERS, 2 * HID, HID], F16, isOutput=False)
    d_bq = nc.declare_dram_parameter("bq", [LAYERS, HID, 1], F32, isOutput=False)
    d_bk = nc.declare_dram_parameter("bk", [LAYERS, HID, 1], F32, isOutput=False)
    d_bv = nc.declare_dram_parameter("bv", [LAYERS, 1, HID], F16, isOutput=False)
    d_b1 = nc.declare_dram_parameter("b1", [LAYERS, 2 * HID, 1], F32, isOutput=False)
    d_g1 = nc.declare_dram_parameter("g1", [LAYERS, HID, 1], F32, isOutput=False)
    d_be1 = nc.declare_dram_parameter("be1", [LAYERS, HID, 1], F32, isOutput=False)
    d_g2 = nc.declare_dram_parameter("g2", [LAYERS, HID, 1], F32, isOutput=False)
    d_be2 = nc.declare_dram_parameter("be2", [LAYERS, HID, 1], F32, isOutput=False)
    d_wh = nc.declare_dram_parameter("wh", [HID, HID], F16, isOutput=False)
    d_wb = nc.declare_dram_parameter("wb", [2 * HID, HID], F16, isOutput=False)
    d_cen = nc.declare_dram_parameter("censel", [HID, 2 * BC], F32, isOutput=False)

    d_gout = nc.declare_dram_parameter("goutT", [HID, BC], F32, isOutput=True)
    d_closs = nc.declare_dram_parameter("closs", [1, 1], F32, isOutput=True)

    with tile.TileContext(nc) as tc:
        with (
            tc.tile_pool(name="consts", bufs=1) as cp,
            tc.tile_pool(name="acts", bufs=2) as ap_,
            tc.tile_pool(name="qkp", bufs=2) as qkp,
            tc.tile_pool(name="vep", bufs=9) as vep,
            tc.tile_pool(name="eap", bufs=9) as eap,
            tc.tile_pool(name="etp", bufs=2) as etp,
            tc.tile_pool(name="a16p", bufs=2) as a16p,
            tc.tile_pool(name="atp2", bufs=2) as atp2,
            tc.tile_pool(name="ffnp", bufs=2) as fp_,
            tc.tile_pool(name="small", bufs=2) as sp_,
            tc.tile_pool(name="ps", bufs=4, space="PSUM") as pps,
            tc.tile_pool(name="pbig", bufs=2, space="PSUM") as ppb,
            tc.tile_pool(name="dram", bufs=4, space="DRAM") as dp_,
        ):
            sdma = nc.sync.dma_start

            # ---- load constants ----
            id16 = cp.tile([128, 128], F16, tag="id16")
            make_identity(nc, id16[:])

            mask_sb = cp.tile([128, 512], F16, tag="mask")
            sdma(out=mask_sb[:], in_=d_mask[:])

            ones_row = cp.tile([1, 128], F16, tag="ones_row")   # k=1 lhsT, value 1
            nc.vector.memset(ones_row[:], 1.0)
            ones_inv64 = cp.tile([1, 128], F32, tag="ones_i64")  # value 1/64
            nc.vector.memset(ones_inv64[:], 1.0 / NPER)
            ones_col = cp.tile([128, 1], F32, tag="ones_col")    # closs rhs
            nc.vector.memset(ones_col[:], 1.0)
            magic = cp.tile([128, 1], mybir.dt.int32, tag="magic")
            nc.vector.memset(magic[:], 0x5F3759DF)
            negC = cp.tile([128, 1], F32, tag="negC")
            nc.vector.memset(negC[:], -EXPC)
            zbias = cp.tile([128, 1], F32, tag="zbias")
            nc.vector.memset(zbias[:], 0.0)

            def load_w(dram, L, rows, cols, dtype, tag):
                t = cp.tile([128, cols], dtype, tag=tag)
                sdma(out=t[:], in_=dram[L, rows * 128:(rows + 1) * 128, :])
                return t

            wq_sb, wk_sb, wv_sb, wo_sb, w1_sb, w2_sb = [], [], [], [], [], []
            bq_sb, bk_sb, bv_sb, b1_sb = [], [], [], []
            g1_sb, be1_sb, g2_sb, be2_sb = [], [], [], []
            for L in range(LAYERS):
                wq_sb.append([load_w(d_wq, L, kc, HID, F16, f"wq{L}{kc}") for kc in range(2)])
                wk_sb.append([load_w(d_wk, L, kc, HID, F16, f"wk{L}{kc}") for kc in range(2)])
                wv_sb.append([load_w(d_wv, L, kc, HID, F16, f"wv{L}{kc}") for kc in range(2)])
                wo_sb.append([load_w(d_wo, L, kc, HID, F16, f"wo{L}{kc}") for kc in range(2)])
                w1_sb.append([load_w(d_w1, L, kc, 2 * HID, F16, f"w1{L}{kc}") for kc in range(2)])
                w2_sb.append([load_w(d_w2, L, kc, HID, F16, f"w2{L}{kc}") for kc in range(4)])

                def load_b(dram, L, cc, tag):
                    t = cp.tile([128, 1], F32, tag=tag)
                    sdma(out=t[:], in_=dram[L, cc * 128:(cc + 1) * 128, :])
                    return t

                bq_sb.append([load_b(d_bq, L, cc, f"bq{L}{cc}") for cc in range(2)])
                bk_sb.append([load_b(d_bk, L, cc, f"bk{L}{cc}") for cc in range(2)])
                b1_sb.append([load_b(d_b1, L, cc, f"b1{L}{cc}") for cc in range(4)])
                g1_sb.append([load_b(d_g1, L, cc, f"g1{L}{cc}") for cc in range(2)])
                be1_sb.append([load_b(d_be1, L, cc, f"be1{L}{cc}") for cc in range(2)])
                g2_sb.append([load_b(d_g2, L, cc, f"g2{L}{cc}") for cc in range(2)])
                be2_sb.append([load_b(d_be2, L, cc, f"be2{L}{cc}") for cc in range(2)])
                bvt = cp.tile([1, HID], F16, tag=f"bv{L}")
                sdma(out=bvt[:], in_=d_bv[L, :, :])
                bv_sb.append(bvt)

            wh_sb = [load_w(d_wh.rearrange("(o p) c -> o p c", p=128), kc, 0, HID, F16, f"wh{kc}")
                     for kc in range(2)]
            wb_sb = [load_w(d_wb.rearrange("(o p) c -> o p c", p=128), kb, 0, HID, F16, f"wb{kb}")
                     for kb in range(4)]
            cen_sb = []
            for cc in range(2):
                t = cp.tile([128, 2 * BC], F32, tag=f"cen{cc}")
                sdma(out=t[:], in_=d_cen[cc * 128:(cc + 1) * 128, :])
                cen_sb.append(t)

            # ---- initial activations (f32 + fp16 shadow for PE operands) ----
            hcur, h16 = [], []
            for cc in range(2):
                t = ap_.tile([128, NL], F32, tag=f"hc{cc}")
                sdma(out=t[:], in_=d_hT[cc * 128:(cc + 1) * 128, :])
                hcur.append(t)
                t16 = qkp.tile([128, NL], F16, tag=f"h16{cc}", name=f"h16{cc}")
                nc.any.tensor_copy(out=t16[:], in_=t[:])
                h16.append(t16)

            # ---- helpers ----
            def rsqrt_newton(out, x, tmpname):
                """out = 1/sqrt(x), x (128,1) f32, via bit-trick + 2 Newton."""
                xi = x.bitcast(mybir.dt.int32)
                t1 = sp_.tile([128, 1], mybir.dt.int32, tag=tmpname + "i")
                nc.vector.tensor_scalar(out=t1[:], in0=xi, scalar1=1, scalar2=None,
                                        op0=AL.arith_shift_right)
                yi = sp_.tile([128, 1], mybir.dt.int32, tag=tmpname + "y")
                nc.vector.tensor_tensor(out=yi[:], in0=magic[:], in1=t1[:], op=AL.subtract)
                y = yi.bitcast(F32)
                xh = sp_.tile([128, 1], F32, tag=tmpname + "xh")
                nc.vector.tensor_scalar(out=xh[:], in0=x, scalar1=0.5, scalar2=None,
                                        op0=AL.mult)
                for it in range(2):
                    y2 = sp_.tile([128, 1], F32, tag=tmpname + "y2")
                    nc.vector.tensor_tensor(out=y2[:], in0=y, in1=y, op=AL.mult)
                    t2 = sp_.tile([128, 1], F32, tag=tmpname + "t2")
                    nc.vector.tensor_tensor(out=t2[:], in0=y2[:], in1=xh[:], op=AL.mult)
                    u = sp_.tile([128, 1], F32, tag=tmpname + "u")
                    nc.vector.tensor_scalar(out=u[:], in0=t2[:], scalar1=1.5, scalar2=-1.0,
                                            op0=AL.subtract, op1=AL.mult)
                    yn = sp_.tile([128, 1], F32, tag=tmpname + "yn" + str(it))
                    nc.vector.tensor_tensor(out=yn[:], in0=u[:], in1=y, op=AL.mult)
                    y = yn[:]
                nc.vector.tensor_copy(out=out, in_=y)

            def batchnorm(pre, gam, bet, bnname):
                """Global BN over all N nodes. pre: [2 x (128, NL) f32 tiles].
                Returns (scale, bias) per-chunk (128,1) f32 tiles."""
                arin = sp_.tile([128, 4], F32, tag=bnname + "in")
                for cc in range(2):
                    xg = pre[cc][:].rearrange("p (s f) -> p s f", f=512)
                    st = sp_.tile([128, 2, 6], F32, tag=bnname + "st")
                    for s in range(2):
                        nc.vector.bn_stats(out=st[:, s, :], in_=xg[:, s, :])
                    mv = sp_.tile([128, 2], F32, tag=bnname + "mv")
                    nc.vector.bn_aggr(out=mv[:], in_=st[:])
                    # arin[:, 2cc] = mean ; arin[:, 2cc+1] = var + mean^2
                    nc.vector.tensor_copy(out=arin[:, 2 * cc:2 * cc + 1], in_=mv[:, 0:1])
                    m2 = sp_.tile([128, 1], F32, tag=bnname + "m2")
                    nc.vector.tensor_tensor(out=m2[:], in0=mv[:, 0:1], in1=mv[:, 0:1],
                                            op=AL.mult)
                    nc.vector.tensor_tensor(out=arin[:, 2 * cc + 1:2 * cc + 2],
                                            in0=mv[:, 1:2], in1=m2[:], op=AL.add)
                bin_ = dp_.tile([128, 4], F32, tag=bnname + "bi")
                bout = dp_.tile([128, 4], F32, tag=bnname + "bo")
                sdma(out=bin_[:], in_=arin[:])
                nc.gpsimd.collective_compute(
                    "AllReduce", AL.add,
                    replica_groups=[list(range(NCORES))],
                    ins=[bin_.opt()], outs=[bout.opt()],
                )
                arout = sp_.tile([128, 4], F32, tag=bnname + "out")
                sdma(out=arout[:], in_=bout[:])
                scales, biases = [], []
                for cc in range(2):
                    gm = sp_.tile([128, 1], F32, tag=bnname + f"gm{cc}")
                    nc.vector.tensor_scalar(out=gm[:], in0=arout[:, 2 * cc:2 * cc + 1],
                                            scalar1=1.0 / NCORES, scalar2=None, op0=AL.mult)
                    gv = sp_.tile([128, 1], F32, tag=bnname + f"gv{cc}")
                    nc.vector.tensor_scalar(out=gv[:], in0=arout[:, 2 * cc + 1:2 * cc + 2],
                                            scalar1=1.0 / NCORES, scalar2=EPS,
                                            op0=AL.mult, op1=AL.add)
                    gm2 = sp_.tile([128, 1], F32, tag=bnname + f"gm2{cc}")
                    nc.vector.tensor_tensor(out=gm2[:], in0=gm[:], in1=gm[:], op=AL.mult)
                    nc.vector.tensor_tensor(out=gv[:], in0=gv[:], in1=gm2[:], op=AL.subtract)
                    rstd = sp_.tile([128, 1], F32, tag=bnname + f"rs{cc}")
                    rsqrt_newton(rstd[:], gv[:], bnname + f"nw{cc}")
                    sc = sp_.tile([128, 1], F32, tag=bnname + f"sc{cc}")
                    nc.vector.tensor_tensor(out=sc[:], in0=gam[cc][:], in1=rstd[:], op=AL.mult)
                    t = sp_.tile([128, 1], F32, tag=bnname + f"t{cc}")
                    nc.vector.tensor_tensor(out=t[:], in0=gm[:], in1=sc[:], op=AL.mult)
                    bi = sp_.tile([128, 1], F32, tag=bnname + f"bi{cc}")
                    nc.vector.tensor_tensor(out=bi[:], in0=bet[cc][:], in1=t[:], op=AL.subtract)
                    scales.append(sc)
                    biases.append(bi)
                return scales, biases

            # ================= layers =================
            for L in range(LAYERS):
                # ---- Q, K projections (transposed out, fp16) ----
                q16, k16 = [], []
                for (w_sb, b_sb, dst) in ((wq_sb[L], bq_sb[L], q16),
                                          (wk_sb[L], bk_sb[L], k16)):
                    for cc in range(2):
                        pq = ppb.tile([128, 1024], F32, tag="pbig")
                        for nh in range(2):
                            for kc in range(2):
                                nc.tensor.matmul(
                                    out=pq[:, nh * 512:(nh + 1) * 512],
                                    lhsT=w_sb[kc][:, cc * 128:(cc + 1) * 128],
                                    rhs=h16[kc][:, nh * 512:(nh + 1) * 512],
                                    start=(kc == 0), stop=(kc == 1))
                        t = qkp.tile([128, NL], F16, tag=f"qk{len(dst)}")
                        nc.vector.tensor_scalar(out=t[:], in0=pq[:], scalar1=b_sb[cc][:],
                                                scalar2=None, op0=AL.add)
                        dst.append(t)

                # ---- V projection (node-major) + Vext with ones columns ----
                vext = []
                for p in range(PAIRS):
                    pv = pps.tile([128, 512], F32, tag="ps")
                    for kc in range(2):
                        nc.tensor.matmul(
                            out=pv[:, 0:256],
                            lhsT=h16[kc][:, p * 128:(p + 1) * 128],
                            rhs=wv_sb[L][kc][:],
                            start=(kc == 0), stop=False)
                    nc.tensor.matmul(
                        out=pv[:, 0:256],
                        lhsT=ones_row[:],
                        rhs=bv_sb[L][:],
                        start=False, stop=True)
                    vx = vep.tile([128, HEADS * 33], F16, tag="vext")
                    vxv = vx[:].rearrange("p (h x) -> p h x", x=33)
                    nc.vector.memset(vxv[:, :, 32:33], 1.0)
                    nc.vector.tensor_copy(out=vxv[:, :, 0:32],
                                          in_=pv[:, 0:256].rearrange("p (h x) -> p h x", x=32))
                    vext.append(vx)

                # ---- S^T per head: matmuls, exp, mask ----
                ea16 = []
                for h in range(HEADS):
                    hc, hr = h // 4, 32 * (h % 4)
                    psx = pps.tile([128, 512], F32, tag="ps")
                    for g in range(BC):
                        half, blk = g % 2, g // 2
                        nc.tensor.matmul(
                            out=psx[64 * half:64 * half + 64, 64 * blk:64 * blk + 64],
                            lhsT=k16[hc][hr:hr + 32, 64 * g:64 * g + 64],
                            rhs=q16[hc][hr:hr + 32, 64 * g:64 * g + 64],
                            start=True, stop=True,
                            tile_position=(hr, 64 * half))
                    et = etp.tile([128, 512], F16, tag="et")
                    nc.scalar.activation(out=et[:], in_=psx[:],
                                         func=mybir.ActivationFunctionType.Exp,
                                         bias=negC[:], scale=1.0)
                    ea = eap.tile([128, 512], F16, tag="ea")
                    nc.vector.tensor_tensor(out=ea[:], in0=et[:], in1=mask_sb[:], op=AL.mult)
                    ea16.append(ea)

                # ---- AV (with appended ones column -> row sums), normalize,
                # ---- transpose back to channel-major ----
                attnT = [atp2.tile([128, NL], F16, tag=f"attnT{cc}", name=f"attnT{cc}")
                         for cc in range(2)]
                for p in range(PAIRS):
                    pav = pps.tile([128, 512], F32, tag="ps")
                    pavv = pav[:, 0:264].rearrange("p (h x) -> p h x", x=33)
                    for h in range(HEADS):
                        for half in range(2):
                            nc.tensor.matmul(
                                out=pav[64 * half:64 * half + 64, 33 * h:33 * h + 33],
                                lhsT=ea16[h][64 * half:64 * half + 64, 64 * p:64 * p + 64],
                                rhs=vext[p][64 * half:64 * half + 64, 33 * h:33 * h + 33],
                                start=True, stop=True,
                                tile_position=(64 * half, 64 * half))
                    rpi = sp_.tile([128, 8], F32, tag="rpi")
                    nc.vector.reciprocal(out=rpi[:], in_=pavv[:, :, 32])
                    a16 = a16p.tile([128, 256], F16, tag="a16")
                    rb32 = _ap_append(rpi[:], 0, 32)
                    nc.vector.tensor_tensor(
                        out=a16[:].rearrange("p (h x) -> p h x", x=32),
                        in0=pavv[:, :, 0:32],
                        in1=rb32, op=AL.mult)
                    pt = pps.tile([128, 512], F16, tag="ps", name="pt")
                    for cc in range(2):
                        nc.tensor.transpose(out=pt[:, 128 * cc:128 * cc + 128],
                                            in_=a16[:, 128 * cc:128 * cc + 128],
                                            identity=id16[:])
                        nc.any.tensor_copy(out=attnT[cc][:, 128 * p:128 * p + 128],
                                           in_=pt[:, 128 * cc:128 * cc + 128])

                # ---- O projection + residual ----
                hres = []
                for cc in range(2):
                    po = ppb.tile([128, 1024], F32, tag="pbig")
                    for nh in range(2):
                        for kc in range(2):
                            nc.tensor.matmul(
                                out=po[:, nh * 512:(nh + 1) * 512],
                                lhsT=wo_sb[L][kc][:, cc * 128:(cc + 1) * 128],
                                rhs=attnT[kc][:, nh * 512:(nh + 1) * 512],
                                start=(kc == 0), stop=(kc == 1))
                    hr_ = ap_.tile([128, NL], F32, tag=f"res{cc}")
                    nc.vector.tensor_tensor(out=hr_[:], in0=po[:], in1=hcur[cc][:], op=AL.add)
                    hres.append(hr_)

                # ---- BN1 + GELU ----
                sc1, bi1 = batchnorm(hres, g1_sb[L], be1_sb[L], f"bn1_{L}")
                x2, x216 = [], []
                for cc in range(2):
                    t = ap_.tile([128, NL], F32, tag=f"x2{cc}")
                    nc.scalar.activation(out=t[:], in_=hres[cc][:],
                                         func=mybir.ActivationFunctionType.Gelu,
                                         bias=bi1[cc][:], scale=sc1[cc][:])
                    x2.append(t)
                    t16 = qkp.tile([128, NL], F16, tag=f"x216{cc}", name=f"x216{cc}")
                    nc.any.tensor_copy(out=t16[:], in_=t[:])
                    x216.append(t16)

                # ---- FFN ----
                ffn = []
                for m4 in range(4):
                    pf = ppb.tile([128, 1024], F32, tag="pbig")
                    for nh in range(2):
                        for kc in range(2):
                            nc.tensor.matmul(
                                out=pf[:, nh * 512:(nh + 1) * 512],
                                lhsT=w1_sb[L][kc][:, m4 * 128:(m4 + 1) * 128],
                                rhs=x216[kc][:, nh * 512:(nh + 1) * 512],
                                start=(kc == 0), stop=(kc == 1))
                    t = fp_.tile([128, NL], F16, tag=f"ffn{m4}")
                    nc.scalar.activation(out=t[:], in_=pf[:],
                                         func=mybir.ActivationFunctionType.Gelu,
                                         bias=b1_sb[L][m4][:], scale=1.0)
                    ffn.append(t)

                pre2 = []
                for cc in range(2):
                    pf2 = ppb.tile([128, 1024], F32, tag="pbig")
                    for nh in range(2):
                        for kc4 in range(4):
                            nc.tensor.matmul(
                                out=pf2[:, nh * 512:(nh + 1) * 512],
                                lhsT=w2_sb[L][kc4][:, cc * 128:(cc + 1) * 128],
                                rhs=ffn[kc4][:, nh * 512:(nh + 1) * 512],
                                start=(kc4 == 0), stop=(kc4 == 3))
                    t = ap_.tile([128, NL], F32, tag=f"res{cc}")
                    nc.vector.tensor_tensor(out=t[:], in0=pf2[:], in1=x2[cc][:], op=AL.add)
                    pre2.append(t)

                # ---- BN2 ----
                sc2, bi2 = batchnorm(pre2, g2_sb[L], be2_sb[L], f"bn2_{L}")
                hnew, h16new = [], []
                for cc in range(2):
                    t = ap_.tile([128, NL], F32, tag=f"hc{cc}")
                    nc.vector.tensor_scalar(out=t[:], in0=pre2[cc][:], scalar1=sc2[cc][:],
                                            scalar2=bi2[cc][:], op0=AL.mult, op1=AL.add)
                    hnew.append(t)
                    t16 = qkp.tile([128, NL], F16, tag=f"h16{cc}", name=f"h16n{cc}")
                    nc.scalar.activation(out=t16[:], in_=pre2[cc][:],
                                         func=mybir.ActivationFunctionType.Identity,
                                         bias=bi2[cc][:], scale=sc2[cc][:])
                    h16new.append(t16)
                hcur, h16 = hnew, h16new

            # ================= readout =================
            # hw = h @ W_h (channel-major), cast fp16
            hw16 = []
            for cc in range(2):
                ph = ppb.tile([128, 1024], F32, tag="pbig")
                for nh in range(2):
                    for kc in range(2):
                        nc.tensor.matmul(
                            out=ph[:, nh * 512:(nh + 1) * 512],
                            lhsT=wh_sb[kc][:, cc * 128:(cc + 1) * 128],
                            rhs=h16[kc][:, nh * 512:(nh + 1) * 512],
                            start=(kc == 0), stop=(kc == 1))
                t = sp_.tile([128, NL], F16, tag=f"hw16{cc}")
                nc.any.tensor_copy(out=t[:], in_=ph[:])
                hw16.append(t)

            # base vectors: columns head (v=0) and tail (v=1) of each graph
            bv4 = []
            for part, v in ((0, 0), (1, 0), (0, 1), (1, 1)):
                # order: [h_head chunk0, h_head chunk1, h_tail chunk0, h_tail chunk1]
                t = sp_.tile([128, BC], F16, tag=f"bv4_{len(bv4)}")
                hv = hcur[part][:].rearrange("p (g v) -> p g v", v=NPER)
                nc.vector.tensor_copy(out=t[:], in_=hv[:, :, v])
                bv4.append(t)

            tb16 = []
            for m in range(2):
                pt = pps.tile([128, 512], F32, tag="ps")
                for kb in range(4):
                    nc.tensor.matmul(
                        out=pt[:, 0:BC],
                        lhsT=wb_sb[kb][:, m * 128:(m + 1) * 128],
                        rhs=bv4[kb][:],
                        start=(kb == 0), stop=(kb == 3))
                t = sp_.tile([128, BC], F16, tag=f"tb16{m}")
                nc.any.tensor_copy(out=t[:], in_=pt[:, 0:BC])
                tb16.append(t)

            # per-node scores s[g, n] via per-graph (1 x 64) matmuls
            ps4 = pps.tile([128, 512], F32, tag="ps")
            for g in range(BC):
                q, blk = g // 4, g % 4
                for cc in range(2):
                    # lhsT column replicated 32x (stride-0) so the matmul
                    # fills a whole 32-row group: no unwritten-psum reads.
                    nc.tensor.matmul(
                        out=ps4[32 * q:32 * q + 32, 64 * blk:64 * blk + 64],
                        lhsT=_ap_append(tb16[cc][:, g:g + 1], 0, 32),
                        rhs=hw16[cc][:, 64 * g:64 * g + 64],
                        start=(cc == 0), stop=(cc == 1),
                        tile_position=(0, 32 * q))
            # softmax in the sparse (rows 0/32/64/96) layout; DVE cost only
            # depends on the free size, and the garbage rows are never read.
            s_sb = sp_.tile([128, 256], F32, tag="s_sb")
            nc.vector.tensor_copy(out=s_sb[:], in_=ps4[:, 0:256])
            sv = s_sb[:].rearrange("p (b f) -> p b f", f=64)
            negm = sp_.tile([128, 4], F32, tag="negm")
            nc.vector.tensor_reduce(out=negm[:], in_=sv, axis=mybir.AxisListType.X,
                                    op=AL.max, negate=True)
            e_sb = sp_.tile([128, 256], F32, tag="e_sb")
            ev = e_sb[:].rearrange("p (b f) -> p b f", f=64)
            nc.vector.tensor_tensor(out=ev, in0=sv, in1=_ap_append(negm[:], 0, 64),
                                    op=AL.add)
            nc.scalar.activation(out=e_sb[:], in_=e_sb[:],
                                 func=mybir.ActivationFunctionType.Exp,
                                 bias=zbias[:], scale=1.0)
            se = sp_.tile([128, 4], F32, tag="se")
            nc.vector.tensor_reduce(out=se[:], in_=ev, axis=mybir.AxisListType.X,
                                    op=AL.add)
            ri = sp_.tile([128, 4], F32, tag="ri")
            nc.vector.reciprocal(out=ri[:], in_=se[:])
            a_sb = sp_.tile([128, 256], F32, tag="a_sb")
            nc.vector.tensor_tensor(out=a_sb[:].rearrange("p (b f) -> p b f", f=64),
                                    in0=ev, in1=_ap_append(ri[:], 0, 64), op=AL.mult)
            a_row = sp_.tile([1, NL], F32, tag="a_row")
            _pitch = a_sb[:].ap[0][0]
            a_gather = bass.AP(tensor=a_sb[:].tensor, offset=a_sb[:].offset,
                               ap=[[_pitch * 32, 4], [64, 4], [1, 64]])
            sdma(out=a_row[:], in_=a_gather)

            pab = ppb.tile([128, 1024], F32, tag="pbig")
            for nh in range(2):
                nc.tensor.matmul(out=pab[:, nh * 512:(nh + 1) * 512],
                                 lhsT=ones_inv64[:],
                                 rhs=a_row[:, nh * 512:(nh + 1) * 512],
                                 start=True, stop=True)
            for cc in range(2):
                pr = sp_.tile([128, NL], F32, tag=f"pr{cc}")
                nc.vector.tensor_tensor(out=pr[:], in0=hcur[cc][:], in1=pab[:], op=AL.mult)
                gsb = sp_.tile([128, BC], F32, tag=f"gsb{cc}")
                nc.vector.tensor_reduce(out=gsb[:], in_=pr[:].rearrange("p (g v) -> p g v", v=NPER),
                                        axis=mybir.AxisListType.X, op=AL.add)
                sdma(out=d_gout[cc * 128:(cc + 1) * 128, :], in_=gsb[:])

            # ---- center loss partial ----
            pcl = pps.tile([128, 512], F32, tag="ps")
            for cc in range(2):
                fh = sp_.tile([128, 2 * BC], F32, tag=f"fh{cc}")
                hv = hcur[cc][:].rearrange("p (g v) -> p g v", v=NPER)
                nc.vector.tensor_copy(out=fh[:, 0:BC], in_=hv[:, :, 0])
                nc.vector.tensor_copy(out=fh[:, BC:2 * BC], in_=hv[:, :, 1])
                d_ = sp_.tile([128, 2 * BC], F32, tag=f"d{cc}")
                nc.vector.tensor_tensor(out=d_[:], in0=fh[:], in1=cen_sb[cc][:],
                                        op=AL.subtract)
                dj = sp_.tile([128, 2 * BC], F32, tag=f"dj{cc}")
                dsum = sp_.tile([128, 1], F32, tag=f"dsum{cc}")
                nc.scalar.activation(out=dj[:], in_=d_[:],
                                     func=mybir.ActivationFunctionType.Square,
                                     bias=zbias[0:128, :], accum_out=dsum[:])
                nc.tensor.matmul(out=pcl[0:1, 0:1], lhsT=dsum[:], rhs=ones_col[:],
                                 start=(cc == 0), stop=(cc == 1))
            cl_sb = sp_.tile([1, 1], F32, tag="cl")
            nc.vector.tensor_copy(out=cl_sb[:], in_=pcl[0:1, 0:1])
            sdma(out=d_closs[:], in_=cl_sb[:])

    nc.compile()
    return nc


def prep_inputs(h, params, row, col, head_ids, tail_ids, drug_pairs):
    """Build the 8 per-core input maps."""
    h = np.asarray(h, np.float32)
    row = np.asarray(row); col = np.asarray(col)
    head_ids = np.asarray(head_ids); tail_ids = np.asarray(tail_ids)
    drug_pairs = np.asarray(drug_pairs)
    assert np.array_equal(head_ids, np.arange(B, dtype=head_ids.dtype) * NPER), \
        "unexpected head_ids layout"
    assert np.array_equal(tail_ids, head_ids + 1), "unexpected tail_ids layout"
    assert np.all(col // NPER == row // NPER), "edges cross graphs"

    # multiplicity mask
    M = np.zeros((B, NPER, NPER), np.float32)
    np.add.at(M, (row // NPER, row % NPER, col % NPER), 1.0)

    layers = params["layers"]
    wq = np.stack([np.asarray(p["q"][0], np.float32)[:, PERM] * np.float32(SCALE)
                   for p in layers]).astype(np.float16)
    wk = np.stack([np.asarray(p["k"][0], np.float32)[:, PERM] for p in layers]).astype(np.float16)
    wv = np.stack([np.asarray(p["v"][0], np.float32)[:, PERM] for p in layers]).astype(np.float16)
    wo = np.stack([np.asarray(p["o"][0], np.float32)[PERM, :] for p in layers]).astype(np.float16)
    w1 = np.stack([np.asarray(p["f1"][0], np.float32) for p in layers]).astype(np.float16)
    w2 = np.stack([np.asarray(p["f2"][0], np.float32) for p in layers]).astype(np.float16)
    bq = np.stack([np.asarray(p["q"][1], np.float32)[PERM] * np.float32(SCALE)
                   for p in layers]).reshape(LAYERS, HID, 1)
    bk = np.stack([np.asarray(p["k"][1], np.float32)[PERM] for p in layers]).reshape(LAYERS, HID, 1)
    bv = np.stack([np.asarray(p["v"][1], np.float32)[PERM] for p in layers]).reshape(LAYERS, 1, HID).astype(np.float16)
    b1 = np.stack([np.asarray(p["f1"][1], np.float32) for p in layers]).reshape(LAYERS, 2 * HID, 1)
    g1 = np.stack([np.asarray(p["bn1"][0], np.float32) for p in layers]).reshape(LAYERS, HID, 1)
    be1 = np.stack([np.asarray(p["bn1"][1], np.float32) for p in layers]).reshape(LAYERS, HID, 1)
    g2 = np.stack([np.asarray(p["bn2"][0], np.float32) for p in layers]).reshape(LAYERS, HID, 1)
    be2 = np.stack([np.asarray(p["bn2"][1], np.float32) for p in layers]).reshape(LAYERS, HID, 1)
    wh = np.asarray(params["W_h"], np.float32).astype(np.float16)
    wb = np.asarray(params["W_base"], np.float32).astype(np.float16)
    centers = np.asarray(params["centers"], np.float32)

    shared = dict(wq=wq, wk=wk, wv=wv, wo=wo, w1=w1, w2=w2, bq=bq, bk=bk, bv=bv,
                  b1=b1, g1=g1, be1=be1, g2=g2, be2=be2, wh=wh, wb=wb)
    shared = {k: np.ascontiguousarray(v) for k, v in shared.items()}

    in_maps = []
    for c in range(NCORES):
        hT = np.ascontiguousarray(h[c * NL:(c + 1) * NL].T)
        maskT = np.zeros((128, 512), np.float16)
        Mc = M[c * BC:(c + 1) * BC]
        for g in range(BC):
            half, blk = g % 2, g // 2
            maskT[64 * half:64 * half + 64, 64 * blk:64 * blk + 64] = Mc[g].T
        censel = np.zeros((HID, 2 * BC), np.float32)
        for g in range(BC):
            censel[:, g] = centers[drug_pairs[c * BC + g, 0]]
            censel[:, BC + g] = centers[drug_pairs[c * BC + g, 1]]
        m = dict(hT=hT, maskT=maskT, censel=np.ascontiguousarray(censel))
        m.update(shared)
        in_maps.append(m)
    return in_maps


_PROGRAM_CACHE = {}


def kernel(h, params, row, col, graph_id, head_ids, tail_ids, drug_pairs,
           _want_time=False):
    in_maps = prep_inputs(h, params, row, col, head_ids, tail_ids, drug_pairs)
    if "nc" not in _PROGRAM_CACHE:
        _PROGRAM_CACHE["nc"] = build_program()
    nc = _PROGRAM_CACHE["nc"]
    res = run_bass_kernel_spmd(nc, in_maps, list(range(NCORES)),
                               trace=_want_time)
    g_out = np.concatenate([res.results[c]["goutT"].T for c in range(NCORES)], axis=0)
    closs = np.float32(sum(float(res.results[c]["closs"][0, 0])
                           for c in range(NCORES)) / (2 * B))
    if _want_time:
        return (g_out.astype(np.float32), closs), res
    return g_out.astype(np.float32), closs


# revision 38
# speedup vs baseline: 1.1039x; 1.1039x over previous
"""Trainium2 Bass kernel for nn_BKG_encoder (sparse graph-transformer encoder).

Strategy:
- Pure data parallelism: 16 of the 128 independent 64-node subgraphs per core.
- Sparse attention (fixed 16 out-edges per node, edges stay inside each
  64-node graph) is recomputed as dense 64x64 masked attention, where the
  mask is the edge multiplicity matrix (built host-side from row/col).
  exp(S - C) * mult, row-normalized, is mathematically identical to the
  reference's edge softmax.
- Activations are stored channel-on-partition ("transposed", hT = (256,
  1024-local-nodes)) so projections/FFN/BN are natural; attention S^T is
  computed per head with 64x64 graph blocks packed 2-per-128-partitions.
- BatchNorm is exact: per-core sums go through a tiny (128,4) AllReduce.
- fp16 is used only inside attention (q/k/v/exp/attn); everything dense
  runs fp32r on the PE (full speed at free-dim >= 256) with fp32 storage.
"""
import os
import sys

for _p in ("/opt/trn_rl_repo", "/opt/pypackages"):
    if _p not in sys.path:
        sys.path.insert(0, _p)

import numpy as np

import concourse.bass as bass
import concourse.bacc as bacc
import concourse.tile as tile
from concourse import mybir
from concourse.masks import make_identity
from concourse.bass_utils import run_bass_kernel_spmd

F32 = mybir.dt.float32
F32R = mybir.dt.float32r
F16 = mybir.dt.float16

HID = 256
HEADS = 8
HD = HID // HEADS          # 32
LAYERS = 3
N = 8192
B = 128
NPER = N // B              # 64
DEG = 16
EPS = 1e-5
SCALE = HD ** -0.5

NCORES = 8
BC = B // NCORES           # 16 graphs per core
NL = N // NCORES           # 1024 nodes per core
PAIRS = NL // 128          # 8 pairs of graphs (128-node blocks)
EXPC = 6.0                 # constant subtracted inside exp for fp16 range

# head-major channel permutation: PERM[32h + d] = d*HEADS + h
PERM = np.array([d * HEADS + h for h in range(HEADS) for d in range(HD)], np.int64)

AL = mybir.AluOpType


def r32(ap):
    return ap.bitcast(F32R)


def _ap_append(ap, stride, size):
    """Append an innermost free dim [stride, size] to an AP."""
    return bass.AP(tensor=ap.tensor, offset=ap.offset, ap=[*ap.ap, [stride, size]])


def build_program():
    nc = bacc.Bacc("TRN2", target_bir_lowering=False, debug=False,
                   num_devices=NCORES)

    # ---- DRAM parameters (per-core inputs) ----
    d_hT = nc.declare_dram_parameter("hT", [HID, NL], F32, isOutput=False)
    d_mask = nc.declare_dram_parameter("maskT", [128, 512], F16, isOutput=False)
    d_wq = nc.declare_dram_parameter("wq", [LAYERS, HID, HID], F16, isOutput=False)
    d_wk = nc.declare_dram_parameter("wk", [LAYERS, HID, HID], F16, isOutput=False)
    d_wv = nc.declare_dram_parameter("wv", [LAYERS, HID, HID], F16, isOutput=False)
    d_wo = nc.declare_dram_parameter("wo", [LAYERS, HID, HID], F16, isOutput=False)
    d_w1 = nc.declare_dram_parameter("w1", [LAYERS, HID, 2 * HID], F16, isOutput=False)
    d_w2 = nc.declare_dram_parameter("w2", [LAYERS, 2 * HID, HID], F16, isOutput=False)
    d_bq = nc.declare_dram_parameter("bq", [LAYERS, HID, 1], F32, isOutput=False)
    d_bk = nc.declare_dram_parameter("bk", [LAYERS, HID, 1], F32, isOutput=False)
    d_bv = nc.declare_dram_parameter("bv", [LAYERS, 1, HID], F16, isOutput=False)
    d_b1 = nc.declare_dram_parameter("b1", [LAYERS, 2 * HID, 1], F32, isOutput=False)
    d_g1 = nc.declare_dram_parameter("g1", [LAYERS, HID, 1], F32, isOutput=False)
    d_be1 = nc.declare_dram_parameter("be1", [LAYERS, HID, 1], F32, isOutput=False)
    d_g2 = nc.declare_dram_parameter("g2", [LAYERS, HID, 1], F32, isOutput=False)
    d_be2 = nc.declare_dram_parameter("be2", [LAYERS, HID, 1], F32, isOutput=False)
    d_wh = nc.declare_dram_parameter("wh", [HID, HID], F16, isOutput=False)
    d_wb = nc.declare_dram_parameter("wb", [2 * HID, HID], F16, isOutput=False)
    d_cen = nc.declare_dram_parameter("censel", [HID, 2 * BC], F32, isOutput=False)

    d_gout = nc.declare_dram_parameter("goutT", [HID, BC], F32, isOutput=True)
    d_closs = nc.declare_dram_parameter("closs", [1, 1], F32, isOutput=True)

    with tile.TileContext(nc) as tc:
        with (
            tc.tile_pool(name="consts", bufs=1) as cp,
            tc.tile_pool(name="acts", bufs=2) as ap_,
            tc.tile_pool(name="qkp", bufs=2) as qkp,
            tc.tile_pool(name="vep", bufs=9) as vep,
            tc.tile_pool(name="eap", bufs=5) as eap,
            tc.tile_pool(name="etp", bufs=2) as etp,
            tc.tile_pool(name="a16p", bufs=2) as a16p,
            tc.tile_pool(name="atp2", bufs=2) as atp2,
            tc.tile_pool(name="ffnp", bufs=2) as fp_,
            tc.tile_pool(name="small", bufs=2) as sp_,
            tc.tile_pool(name="ps", bufs=4, space="PSUM") as pps,
            tc.tile_pool(name="pbig", bufs=2, space="PSUM") as ppb,
            tc.tile_pool(name="dram", bufs=4, space="DRAM") as dp_,
        ):
            sdma = nc.sync.dma_start

            # ---- load constants ----
            id16 = cp.tile([128, 128], F16, tag="id16")
            make_identity(nc, id16[:])

            mask_sb = cp.tile([128, 512], F16, tag="mask")
            sdma(out=mask_sb[:], in_=d_mask[:])

            ones_row = cp.tile([1, 128], F16, tag="ones_row")   # k=1 lhsT, value 1
            nc.vector.memset(ones_row[:], 1.0)
            ones_inv64 = cp.tile([1, 128], F32, tag="ones_i64")  # value 1/64
            nc.vector.memset(ones_inv64[:], 1.0 / NPER)
            ones_col = cp.tile([128, 1], F32, tag="ones_col")    # closs rhs
            nc.vector.memset(ones_col[:], 1.0)
            magic = cp.tile([128, 1], mybir.dt.int32, tag="magic")
            nc.vector.memset(magic[:], 0x5F3759DF)
            negC = cp.tile([128, 1], F32, tag="negC")
            nc.vector.memset(negC[:], -EXPC)
            zbias = cp.tile([128, 1], F32, tag="zbias")
            nc.vector.memset(zbias[:], 0.0)

            # ---- batched weight loads: one DMA per class, spread over
            # ---- multiple engines' DMA queues ----
            _dmaeng = [nc.sync, nc.scalar, nc.gpsimd]
            _ei = [0]

            def bdma(out, in_):
                e = _dmaeng[_ei[0] % len(_dmaeng)]
                _ei[0] += 1
                e.dma_start(out=out, in_=in_)

            def load_mat(dram, nchunk, cols, dtype, tag):
                """dram (LAYERS, nchunk*128, cols) -> sbuf (128, L*nchunk*cols),
                one DMA. Returns [L][chunk] -> (128, cols) view list."""
                t = cp.tile([128, LAYERS * nchunk * cols], dtype, tag=tag, name=tag)
                bdma(t[:].rearrange("p (l k c) -> p l k c", l=LAYERS, k=nchunk),
                     dram.rearrange("l (k p) c -> p l k c", p=128))
                return [[t[:, (L * nchunk + kc) * cols:(L * nchunk + kc + 1) * cols]
                         for kc in range(nchunk)] for L in range(LAYERS)]

            def load_vecs(dram, nchunk, tag):
                """dram (LAYERS, nchunk*128, 1) -> [L][cc] -> (128, 1) views."""
                t = cp.tile([128, LAYERS * nchunk], F32, tag=tag, name=tag)
                bdma(t[:].rearrange("p (l k) -> p l k", l=LAYERS),
                     dram.rearrange("l (k p) c -> p l (k c)", p=128))
                return [[t[:, L * nchunk + cc:L * nchunk + cc + 1]
                         for cc in range(nchunk)] for L in range(LAYERS)]

            wq_sb = load_mat(d_wq, 2, HID, F16, "wq_all")
            wk_sb = load_mat(d_wk, 2, HID, F16, "wk_all")
            wv_sb = load_mat(d_wv, 2, HID, F16, "wv_all")
            wo_sb = load_mat(d_wo, 2, HID, F16, "wo_all")
            w1_sb = load_mat(d_w1, 2, 2 * HID, F16, "w1_all")
            w2_sb = load_mat(d_w2, 4, HID, F16, "w2_all")
            bq_sb = load_vecs(d_bq, 2, "bq_all")
            bk_sb = load_vecs(d_bk, 2, "bk_all")
            b1_sb = load_vecs(d_b1, 4, "b1_all")
            g1_sb = load_vecs(d_g1, 2, "g1_all")
            be1_sb = load_vecs(d_be1, 2, "be1_all")
            g2_sb = load_vecs(d_g2, 2, "g2_all")
            be2_sb = load_vecs(d_be2, 2, "be2_all")

            bv_all = cp.tile([1, LAYERS * HID], F16, tag="bv_all")
            bdma(bv_all[:].rearrange("p (l c) -> p l c", l=LAYERS),
                 d_bv.rearrange("l p c -> p l c"))
            bv_sb = [bv_all[:, L * HID:(L + 1) * HID] for L in range(LAYERS)]

            wh_all = cp.tile([128, 2 * HID], F16, tag="wh_all")
            bdma(wh_all[:].rearrange("p (k c) -> p k c", k=2),
                 d_wh.rearrange("(k p) c -> p k c", p=128))
            wh_sb = [wh_all[:, kc * HID:(kc + 1) * HID] for kc in range(2)]

            wb_all = cp.tile([128, 4 * HID], F16, tag="wb_all")
            bdma(wb_all[:].rearrange("p (k c) -> p k c", k=4),
                 d_wb.rearrange("(k p) c -> p k c", p=128))
            wb_sb = [wb_all[:, kb * HID:(kb + 1) * HID] for kb in range(4)]

            cen_all = cp.tile([128, 4 * BC], F32, tag="cen_all")
            bdma(cen_all[:].rearrange("p (k c) -> p k c", k=2),
                 d_cen.rearrange("(k p) c -> p k c", p=128))
            cen_sb = [cen_all[:, cc * 2 * BC:(cc + 1) * 2 * BC] for cc in range(2)]

            # ---- initial activations (f32 + fp16 shadow for PE operands) ----
            hcur, h16 = [], []
            for cc in range(2):
                t = ap_.tile([128, NL], F32, tag=f"hc{cc}")
                sdma(out=t[:], in_=d_hT[cc * 128:(cc + 1) * 128, :])
                hcur.append(t)
                t16 = qkp.tile([128, NL], F16, tag=f"h16{cc}", name=f"h16{cc}")
                nc.any.tensor_copy(out=t16[:], in_=t[:])
                h16.append(t16)

            # ---- helpers ----
            def rsqrt_newton(out, x, tmpname):
                """out = 1/sqrt(x), x (128,1) f32, via bit-trick + 2 Newton."""
                xi = x.bitcast(mybir.dt.int32)
                t1 = sp_.tile([128, 1], mybir.dt.int32, tag=tmpname + "i")
                nc.vector.tensor_scalar(out=t1[:], in0=xi, scalar1=1, scalar2=None,
                                        op0=AL.arith_shift_right)
                yi = sp_.tile([128, 1], mybir.dt.int32, tag=tmpname + "y")
                nc.vector.tensor_tensor(out=yi[:], in0=magic[:], in1=t1[:], op=AL.subtract)
                y = yi.bitcast(F32)
                xh = sp_.tile([128, 1], F32, tag=tmpname + "xh")
                nc.vector.tensor_scalar(out=xh[:], in0=x, scalar1=0.5, scalar2=None,
                                        op0=AL.mult)
                for it in range(2):
                    y2 = sp_.tile([128, 1], F32, tag=tmpname + "y2")
                    nc.vector.tensor_tensor(out=y2[:], in0=y, in1=y, op=AL.mult)
                    t2 = sp_.tile([128, 1], F32, tag=tmpname + "t2")
                    nc.vector.tensor_tensor(out=t2[:], in0=y2[:], in1=xh[:], op=AL.mult)
                    u = sp_.tile([128, 1], F32, tag=tmpname + "u")
                    nc.vector.tensor_scalar(out=u[:], in0=t2[:], scalar1=1.5, scalar2=-1.0,
                                            op0=AL.subtract, op1=AL.mult)
                    yn = sp_.tile([128, 1], F32, tag=tmpname + "yn" + str(it))
                    nc.vector.tensor_tensor(out=yn[:], in0=u[:], in1=y, op=AL.mult)
                    y = yn[:]
                nc.vector.tensor_copy(out=out, in_=y)

            def batchnorm(pre, gam, bet, bnname):
                """Global BN over all N nodes. pre: [2 x (128, NL) f32 tiles].
                Returns (scale, bias) per-chunk (128,1) f32 tiles."""
                arin = sp_.tile([128, 4], F32, tag=bnname + "in")
                for cc in range(2):
                    xg = pre[cc][:].rearrange("p (s f) -> p s f", f=512)
                    st = sp_.tile([128, 2, 6], F32, tag=bnname + "st")
                    for s in range(2):
                        nc.vector.bn_stats(out=st[:, s, :], in_=xg[:, s, :])
                    mv = sp_.tile([128, 2], F32, tag=bnname + "mv")
                    nc.vector.bn_aggr(out=mv[:], in_=st[:])
                    # arin[:, 2cc] = mean ; arin[:, 2cc+1] = var + mean^2
                    nc.vector.tensor_copy(out=arin[:, 2 * cc:2 * cc + 1], in_=mv[:, 0:1])
                    m2 = sp_.tile([128, 1], F32, tag=bnname + "m2")
                    nc.vector.tensor_tensor(out=m2[:], in0=mv[:, 0:1], in1=mv[:, 0:1],
                                            op=AL.mult)
                    nc.vector.tensor_tensor(out=arin[:, 2 * cc + 1:2 * cc + 2],
                                            in0=mv[:, 1:2], in1=m2[:], op=AL.add)
                bin_ = dp_.tile([128, 4], F32, tag=bnname + "bi")
                bout = dp_.tile([128, 4], F32, tag=bnname + "bo")
                sdma(out=bin_[:], in_=arin[:])
                nc.gpsimd.collective_compute(
                    "AllReduce", AL.add,
                    replica_groups=[list(range(NCORES))],
                    ins=[bin_.opt()], outs=[bout.opt()],
                )
                arout = sp_.tile([128, 4], F32, tag=bnname + "out")
                sdma(out=arout[:], in_=bout[:])
                scales, biases = [], []
                for cc in range(2):
                    gm = sp_.tile([128, 1], F32, tag=bnname + f"gm{cc}")
                    nc.vector.tensor_scalar(out=gm[:], in0=arout[:, 2 * cc:2 * cc + 1],
                                            scalar1=1.0 / NCORES, scalar2=None, op0=AL.mult)
                    gv = sp_.tile([128, 1], F32, tag=bnname + f"gv{cc}")
                    nc.vector.tensor_scalar(out=gv[:], in0=arout[:, 2 * cc + 1:2 * cc + 2],
                                            scalar1=1.0 / NCORES, scalar2=EPS,
                                            op0=AL.mult, op1=AL.add)
                    gm2 = sp_.tile([128, 1], F32, tag=bnname + f"gm2{cc}")
                    nc.vector.tensor_tensor(out=gm2[:], in0=gm[:], in1=gm[:], op=AL.mult)
                    nc.vector.tensor_tensor(out=gv[:], in0=gv[:], in1=gm2[:], op=AL.subtract)
                    rstd = sp_.tile([128, 1], F32, tag=bnname + f"rs{cc}")
                    rsqrt_newton(rstd[:], gv[:], bnname + f"nw{cc}")
                    sc = sp_.tile([128, 1], F32, tag=bnname + f"sc{cc}")
                    nc.vector.tensor_tensor(out=sc[:], in0=gam[cc][:], in1=rstd[:], op=AL.mult)
                    t = sp_.tile([128, 1], F32, tag=bnname + f"t{cc}")
                    nc.vector.tensor_tensor(out=t[:], in0=gm[:], in1=sc[:], op=AL.mult)
                    bi = sp_.tile([128, 1], F32, tag=bnname + f"bi{cc}")
                    nc.vector.tensor_tensor(out=bi[:], in0=bet[cc][:], in1=t[:], op=AL.subtract)
                    scales.append(sc)
                    biases.append(bi)
                return scales, biases

            # ================= layers =================
            for L in range(LAYERS):
                # ---- Q, K projections (transposed out, fp16) ----
                q16, k16 = [], []
                for (w_sb, b_sb, dst) in ((wq_sb[L], bq_sb[L], q16),
                                          (wk_sb[L], bk_sb[L], k16)):
                    for cc in range(2):
                        pq = ppb.tile([128, 1024], F32, tag="pbig")
                        for nh in range(2):
                            for kc in range(2):
                                nc.tensor.matmul(
                                    out=pq[:, nh * 512:(nh + 1) * 512],
                                    lhsT=w_sb[kc][:, cc * 128:(cc + 1) * 128],
                                    rhs=h16[kc][:, nh * 512:(nh + 1) * 512],
                                    start=(kc == 0), stop=(kc == 1))
                        t = qkp.tile([128, NL], F16, tag=f"qk{len(dst)}")
                        nc.vector.tensor_scalar(out=t[:], in0=pq[:], scalar1=b_sb[cc][:],
                                                scalar2=None, op0=AL.add)
                        dst.append(t)

                # ---- V projection (node-major) + Vext2: per head h, cols
                # [66h:66h+33) hold [V_h|1] valid on rows 0:64 (graph 2p) and
                # [66h+33:66h+66) valid on rows 64:128 (graph 2p+1); zeros
                # elsewhere. This lets one full-K (128) AV matmul produce both
                # graphs separated by column. ----
                vext = []
                for p in range(PAIRS):
                    pv = pps.tile([128, 512], F32, tag="ps")
                    for kc in range(2):
                        nc.tensor.matmul(
                            out=pv[:, 0:256],
                            lhsT=h16[kc][:, p * 128:(p + 1) * 128],
                            rhs=wv_sb[L][kc][:],
                            start=(kc == 0), stop=False)
                    nc.tensor.matmul(
                        out=pv[:, 0:256],
                        lhsT=ones_row[:],
                        rhs=bv_sb[L][:],
                        start=False, stop=True)
                    vx = vep.tile([128, HEADS * 66], F16, tag="vext")
                    nc.vector.memset(vx[:], 0.0)
                    vxp = vx[:].ap[0][0]
                    vxoff = vx[:].offset
                    for half in range(2):
                        off = vxoff + half * (64 * vxp + 33)
                        nc.vector.memset(
                            bass.AP(tensor=vx[:].tensor, offset=off + 32,
                                    ap=[[vxp, 64], [66, 8]]), 1.0)
                        nc.vector.tensor_copy(
                            out=bass.AP(tensor=vx[:].tensor, offset=off,
                                        ap=[[vxp, 64], [66, 8], [1, 32]]),
                            in_=pv[64 * half:64 * half + 64, 0:256]
                                .rearrange("p (h x) -> p h x", x=32))
                    vext.append(vx)

                # ---- S^T: 2 heads per psum tile; batched exp + mask ----
                ea16 = []
                mpitch = mask_sb[:].ap[0][0]
                mask2 = bass.AP(tensor=mask_sb[:].tensor, offset=mask_sb[:].offset,
                                ap=[[mpitch, 128], [0, 2], [1, 512]])
                for hp in range(HEADS // 2):
                    psx = ppb.tile([128, 1024], F32, tag="pbig")
                    for hh in range(2):
                        h = 2 * hp + hh
                        hc, hr = h // 4, 32 * (h % 4)
                        for g in range(BC):
                            half, blk = g % 2, g // 2
                            nc.tensor.matmul(
                                out=psx[64 * half:64 * half + 64,
                                        512 * hh + 64 * blk:512 * hh + 64 * blk + 64],
                                lhsT=k16[hc][hr:hr + 32, 64 * g:64 * g + 64],
                                rhs=q16[hc][hr:hr + 32, 64 * g:64 * g + 64],
                                start=True, stop=True,
                                tile_position=(hr, 64 * half))
                    et = etp.tile([128, 1024], F16, tag="et")
                    nc.scalar.activation(out=et[:], in_=psx[:],
                                         func=mybir.ActivationFunctionType.Exp,
                                         bias=negC[:], scale=1.0)
                    ea = eap.tile([128, 1024], F16, tag="ea")
                    nc.vector.tensor_tensor(
                        out=ea[:].rearrange("p (h x) -> p h x", x=512),
                        in0=et[:].rearrange("p (h x) -> p h x", x=512),
                        in1=mask2, op=AL.mult)
                    ea16.append(ea)

                # ---- AV: one full-K matmul per (head, pair); normalize,
                # ---- transpose back to channel-major ----
                attnT = [atp2.tile([128, NL], F16, tag=f"attnT{cc}", name=f"attnT{cc}")
                         for cc in range(2)]
                for p in range(PAIRS):
                    pav = ppb.tile([128, 1024], F32, tag="pbig")
                    ppitch = pav[:].ap[0][0]
                    poff = pav[:].offset
                    for h in range(HEADS):
                        hcol = 512 * (h // 4) + 66 * (h % 4)
                        nc.tensor.matmul(
                            out=pav[0:64, hcol:hcol + 66],
                            lhsT=ea16[h // 2][:, 512 * (h % 2) + 64 * p:
                                              512 * (h % 2) + 64 * p + 64],
                            rhs=vext[p][:, 66 * h:66 * h + 66],
                            start=True, stop=True)
                    rpi = sp_.tile([64, 16], F32, tag="rpi")
                    nc.vector.reciprocal(
                        out=rpi[:].rearrange("p (g m x) -> p g m x", g=2, m=4),
                        in_=bass.AP(tensor=pav[:].tensor, offset=poff + 32,
                                    ap=[[ppitch, 64], [512, 2], [66, 4], [33, 2]]))
                    a16 = a16p.tile([128, 256], F16, tag="a16")
                    for half in range(2):
                        rh = rpi[:].rearrange("p (g m x) -> p g m x", g=2, m=4)[:, :, :, half]
                        nc.vector.tensor_tensor(
                            out=a16[64 * half:64 * half + 64, :]
                                .rearrange("p (g m c) -> p g m c", g=2, m=4),
                            in0=bass.AP(tensor=pav[:].tensor,
                                        offset=poff + 33 * half,
                                        ap=[[ppitch, 64], [512, 2], [66, 4], [1, 32]]),
                            in1=_ap_append(rh, 0, 32), op=AL.mult)
                    pt = pps.tile([128, 528], F16, tag="ps", name="pt")
                    for cc in range(2):
                        nc.tensor.transpose(out=pt[:, 128 * cc:128 * cc + 128],
                                            in_=a16[:, 128 * cc:128 * cc + 128],
                                            identity=id16[:])
                        nc.any.tensor_copy(out=attnT[cc][:, 128 * p:128 * p + 128],
                                           in_=pt[:, 128 * cc:128 * cc + 128])

                # ---- O projection + residual ----
                hres = []
                for cc in range(2):
                    po = ppb.tile([128, 1024], F32, tag="pbig")
                    for nh in range(2):
                        for kc in range(2):
                            nc.tensor.matmul(
                                out=po[:, nh * 512:(nh + 1) * 512],
                                lhsT=wo_sb[L][kc][:, cc * 128:(cc + 1) * 128],
                                rhs=attnT[kc][:, nh * 512:(nh + 1) * 512],
                                start=(kc == 0), stop=(kc == 1))
                    hr_ = ap_.tile([128, NL], F32, tag=f"res{cc}")
                    nc.vector.tensor_tensor(out=hr_[:], in0=po[:], in1=hcur[cc][:], op=AL.add)
                    hres.append(hr_)

                # ---- BN1 + GELU ----
                sc1, bi1 = batchnorm(hres, g1_sb[L], be1_sb[L], f"bn1_{L}")
                x2, x216 = [], []
                for cc in range(2):
                    t = ap_.tile([128, NL], F32, tag=f"x2{cc}")
                    nc.scalar.activation(out=t[:], in_=hres[cc][:],
                                         func=mybir.ActivationFunctionType.Gelu,
                                         bias=bi1[cc][:], scale=sc1[cc][:])
                    x2.append(t)
                    t16 = qkp.tile([128, NL], F16, tag=f"x216{cc}", name=f"x216{cc}")
                    nc.any.tensor_copy(out=t16[:], in_=t[:])
                    x216.append(t16)

                # ---- FFN ----
                ffn = []
                for m4 in range(4):
                    pf = ppb.tile([128, 1024], F32, tag="pbig")
                    for nh in range(2):
                        for kc in range(2):
                            nc.tensor.matmul(
                                out=pf[:, nh * 512:(nh + 1) * 512],
                                lhsT=w1_sb[L][kc][:, m4 * 128:(m4 + 1) * 128],
                                rhs=x216[kc][:, nh * 512:(nh + 1) * 512],
                                start=(kc == 0), stop=(kc == 1))
                    t = fp_.tile([128, NL], F16, tag=f"ffn{m4}")
                    nc.scalar.activation(out=t[:], in_=pf[:],
                                         func=mybir.ActivationFunctionType.Gelu,
                                         bias=b1_sb[L][m4][:], scale=1.0)
                    ffn.append(t)

                pre2 = []
                for cc in range(2):
                    pf2 = ppb.tile([128, 1024], F32, tag="pbig")
                    for nh in range(2):
                        for kc4 in range(4):
                            nc.tensor.matmul(
                                out=pf2[:, nh * 512:(nh + 1) * 512],
                                lhsT=w2_sb[L][kc4][:, cc * 128:(cc + 1) * 128],
                                rhs=ffn[kc4][:, nh * 512:(nh + 1) * 512],
                                start=(kc4 == 0), stop=(kc4 == 3))
                    t = ap_.tile([128, NL], F32, tag=f"res{cc}")
                    nc.vector.tensor_tensor(out=t[:], in0=pf2[:], in1=x2[cc][:], op=AL.add)
                    pre2.append(t)

                # ---- BN2 ----
                sc2, bi2 = batchnorm(pre2, g2_sb[L], be2_sb[L], f"bn2_{L}")
                hnew, h16new = [], []
                for cc in range(2):
                    t = ap_.tile([128, NL], F32, tag=f"hc{cc}")
                    nc.vector.tensor_scalar(out=t[:], in0=pre2[cc][:], scalar1=sc2[cc][:],
                                            scalar2=bi2[cc][:], op0=AL.mult, op1=AL.add)
                    hnew.append(t)
                    t16 = qkp.tile([128, NL], F16, tag=f"h16{cc}", name=f"h16n{cc}")
                    nc.scalar.activation(out=t16[:], in_=pre2[cc][:],
                                         func=mybir.ActivationFunctionType.Identity,
                                         bias=bi2[cc][:], scale=sc2[cc][:])
                    h16new.append(t16)
                hcur, h16 = hnew, h16new

            # ================= readout =================
            # hw = h @ W_h (channel-major), cast fp16
            hw16 = []
            for cc in range(2):
                ph = ppb.tile([128, 1024], F32, tag="pbig")
                for nh in range(2):
                    for kc in range(2):
                        nc.tensor.matmul(
                            out=ph[:, nh * 512:(nh + 1) * 512],
                            lhsT=wh_sb[kc][:, cc * 128:(cc + 1) * 128],
                            rhs=h16[kc][:, nh * 512:(nh + 1) * 512],
                            start=(kc == 0), stop=(kc == 1))
                t = sp_.tile([128, NL], F16, tag=f"hw16{cc}")
                nc.any.tensor_copy(out=t[:], in_=ph[:])
                hw16.append(t)

            # base vectors: columns head (v=0) and tail (v=1) of each graph
            bv4 = []
            for part, v in ((0, 0), (1, 0), (0, 1), (1, 1)):
                # order: [h_head chunk0, h_head chunk1, h_tail chunk0, h_tail chunk1]
                t = sp_.tile([128, BC], F16, tag=f"bv4_{len(bv4)}")
                hv = hcur[part][:].rearrange("p (g v) -> p g v", v=NPER)
                nc.vector.tensor_copy(out=t[:], in_=hv[:, :, v])
                bv4.append(t)

            tb16 = []
            for m in range(2):
                pt = pps.tile([128, 512], F32, tag="ps")
                for kb in range(4):
                    nc.tensor.matmul(
                        out=pt[:, 0:BC],
                        lhsT=wb_sb[kb][:, m * 128:(m + 1) * 128],
                        rhs=bv4[kb][:],
                        start=(kb == 0), stop=(kb == 3))
                t = sp_.tile([128, BC], F16, tag=f"tb16{m}")
                nc.any.tensor_copy(out=t[:], in_=pt[:, 0:BC])
                tb16.append(t)

            # per-node scores s[g, n] via per-graph (1 x 64) matmuls
            ps4 = pps.tile([128, 512], F32, tag="ps")
            for g in range(BC):
                q, blk = g // 4, g % 4
                for cc in range(2):
                    # lhsT column replicated 32x (stride-0) so the matmul
                    # fills a whole 32-row group: no unwritten-psum reads.
                    nc.tensor.matmul(
                        out=ps4[32 * q:32 * q + 32, 64 * blk:64 * blk + 64],
                        lhsT=_ap_append(tb16[cc][:, g:g + 1], 0, 32),
                        rhs=hw16[cc][:, 64 * g:64 * g + 64],
                        start=(cc == 0), stop=(cc == 1),
                        tile_position=(0, 32 * q))
            # softmax in the sparse (rows 0/32/64/96) layout; DVE cost only
            # depends on the free size, and the garbage rows are never read.
            s_sb = sp_.tile([128, 256], F32, tag="s_sb")
            nc.vector.tensor_copy(out=s_sb[:], in_=ps4[:, 0:256])
            sv = s_sb[:].rearrange("p (b f) -> p b f", f=64)
            negm = sp_.tile([128, 4], F32, tag="negm")
            nc.vector.tensor_reduce(out=negm[:], in_=sv, axis=mybir.AxisListType.X,
                                    op=AL.max, negate=True)
            e_sb = sp_.tile([128, 256], F32, tag="e_sb")
            ev = e_sb[:].rearrange("p (b f) -> p b f", f=64)
            nc.vector.tensor_tensor(out=ev, in0=sv, in1=_ap_append(negm[:], 0, 64),
                                    op=AL.add)
            nc.scalar.activation(out=e_sb[:], in_=e_sb[:],
                                 func=mybir.ActivationFunctionType.Exp,
                                 bias=zbias[:], scale=1.0)
            se = sp_.tile([128, 4], F32, tag="se")
            nc.vector.tensor_reduce(out=se[:], in_=ev, axis=mybir.AxisListType.X,
                                    op=AL.add)
            ri = sp_.tile([128, 4], F32, tag="ri")
            nc.vector.reciprocal(out=ri[:], in_=se[:])
            a_sb = sp_.tile([128, 256], F32, tag="a_sb")
            nc.vector.tensor_tensor(out=a_sb[:].rearrange("p (b f) -> p b f", f=64),
                                    in0=ev, in1=_ap_append(ri[:], 0, 64), op=AL.mult)
            a_row = sp_.tile([1, NL], F32, tag="a_row")
            _pitch = a_sb[:].ap[0][0]
            a_gather = bass.AP(tensor=a_sb[:].tensor, offset=a_sb[:].offset,
                               ap=[[_pitch * 32, 4], [64, 4], [1, 64]])
            sdma(out=a_row[:], in_=a_gather)

            pab = ppb.tile([128, 1024], F32, tag="pbig")
            for nh in range(2):
                nc.tensor.matmul(out=pab[:, nh * 512:(nh + 1) * 512],
                                 lhsT=ones_inv64[:],
                                 rhs=a_row[:, nh * 512:(nh + 1) * 512],
                                 start=True, stop=True)
            for cc in range(2):
                pr = sp_.tile([128, NL], F32, tag=f"pr{cc}")
                nc.vector.tensor_tensor(out=pr[:], in0=hcur[cc][:], in1=pab[:], op=AL.mult)
                gsb = sp_.tile([128, BC], F32, tag=f"gsb{cc}")
                nc.vector.tensor_reduce(out=gsb[:], in_=pr[:].rearrange("p (g v) -> p g v", v=NPER),
                                        axis=mybir.AxisListType.X, op=AL.add)
                sdma(out=d_gout[cc * 128:(cc + 1) * 128, :], in_=gsb[:])

            # ---- center loss partial ----
            pcl = pps.tile([128, 512], F32, tag="ps")
            for cc in range(2):
                fh = sp_.tile([128, 2 * BC], F32, tag=f"fh{cc}")
                hv = hcur[cc][:].rearrange("p (g v) -> p g v", v=NPER)
                nc.vector.tensor_copy(out=fh[:, 0:BC], in_=hv[:, :, 0])
                nc.vector.tensor_copy(out=fh[:, BC:2 * BC], in_=hv[:, :, 1])
                d_ = sp_.tile([128, 2 * BC], F32, tag=f"d{cc}")
                nc.vector.tensor_tensor(out=d_[:], in0=fh[:], in1=cen_sb[cc][:],
                                        op=AL.subtract)
                dj = sp_.tile([128, 2 * BC], F32, tag=f"dj{cc}")
                dsum = sp_.tile([128, 1], F32, tag=f"dsum{cc}")
                nc.scalar.activation(out=dj[:], in_=d_[:],
                                     func=mybir.ActivationFunctionType.Square,
                                     bias=zbias[0:128, :], accum_out=dsum[:])
                nc.tensor.matmul(out=pcl[0:1, 0:1], lhsT=dsum[:], rhs=ones_col[:],
                                 start=(cc == 0), stop=(cc == 1))
            cl_sb = sp_.tile([1, 1], F32, tag="cl")
            nc.vector.tensor_copy(out=cl_sb[:], in_=pcl[0:1, 0:1])
            sdma(out=d_closs[:], in_=cl_sb[:])

    nc.compile()
    return nc


def prep_inputs(h, params, row, col, head_ids, tail_ids, drug_pairs):
    """Build the 8 per-core input maps."""
    h = np.asarray(h, np.float32)
    row = np.asarray(row); col = np.asarray(col)
    head_ids = np.asarray(head_ids); tail_ids = np.asarray(tail_ids)
    drug_pairs = np.asarray(drug_pairs)
    assert np.array_equal(head_ids, np.arange(B, dtype=head_ids.dtype) * NPER), \
        "unexpected head_ids layout"
    assert np.array_equal(tail_ids, head_ids + 1), "unexpected tail_ids layout"
    assert np.all(col // NPER == row // NPER), "edges cross graphs"

    # multiplicity mask
    M = np.zeros((B, NPER, NPER), np.float32)
    np.add.at(M, (row // NPER, row % NPER, col % NPER), 1.0)

    layers = params["layers"]
    wq = np.stack([np.asarray(p["q"][0], np.float32)[:, PERM] * np.float32(SCALE)
                   for p in layers]).astype(np.float16)
    wk = np.stack([np.asarray(p["k"][0], np.float32)[:, PERM] for p in layers]).astype(np.float16)
    wv = np.stack([np.asarray(p["v"][0], np.float32)[:, PERM] for p in layers]).astype(np.float16)
    wo = np.stack([np.asarray(p["o"][0], np.float32)[PERM, :] for p in layers]).astype(np.float16)
    w1 = np.stack([np.asarray(p["f1"][0], np.float32) for p in layers]).astype(np.float16)
    w2 = np.stack([np.asarray(p["f2"][0], np.float32) for p in layers]).astype(np.float16)
    bq = np.stack([np.asarray(p["q"][1], np.float32)[PERM] * np.float32(SCALE)
                   for p in layers]).reshape(LAYERS, HID, 1)
    bk = np.stack([np.asarray(p["k"][1], np.float32)[PERM] for p in layers]).reshape(LAYERS, HID, 1)
    bv = np.stack([np.asarray(p["v"][1], np.float32)[PERM] for p in layers]).reshape(LAYERS, 1, HID).astype(np.float16)
    b1 = np.stack([np.asarray(p["f1"][1], np.float32) for p in layers]).reshape(LAYERS, 2 * HID, 1)
    g1 = np.stack([np.asarray(p["bn1"][0], np.float32) for p in layers]).reshape(LAYERS, HID, 1)
    be1 = np.stack([np.asarray(p["bn1"][1], np.float32) for p in layers]).reshape(LAYERS, HID, 1)
    g2 = np.stack([np.asarray(p["bn2"][0], np.float32) for p in layers]).reshape(LAYERS, HID, 1)
    be2 = np.stack([np.asarray(p["bn2"][1], np.float32) for p in layers]).reshape(LAYERS, HID, 1)
    wh = np.asarray(params["W_h"], np.float32).astype(np.float16)
    wb = np.asarray(params["W_base"], np.float32).astype(np.float16)
    centers = np.asarray(params["centers"], np.float32)

    shared = dict(wq=wq, wk=wk, wv=wv, wo=wo, w1=w1, w2=w2, bq=bq, bk=bk, bv=bv,
                  b1=b1, g1=g1, be1=be1, g2=g2, be2=be2, wh=wh, wb=wb)
    shared = {k: np.ascontiguousarray(v) for k, v in shared.items()}

    in_maps = []
    for c in range(NCORES):
        hT = np.ascontiguousarray(h[c * NL:(c + 1) * NL].T)
        maskT = np.zeros((128, 512), np.float16)
        Mc = M[c * BC:(c + 1) * BC]
        for g in range(BC):
            half, blk = g % 2, g // 2
            maskT[64 * half:64 * half + 64, 64 * blk:64 * blk + 64] = Mc[g].T
        censel = np.zeros((HID, 2 * BC), np.float32)
        for g in range(BC):
            censel[:, g] = centers[drug_pairs[c * BC + g, 0]]
            censel[:, BC + g] = centers[drug_pairs[c * BC + g, 1]]
        m = dict(hT=hT, maskT=maskT, censel=np.ascontiguousarray(censel))
        m.update(shared)
        in_maps.append(m)
    return in_maps


_PROGRAM_CACHE = {}


def kernel(h, params, row, col, graph_id, head_ids, tail_ids, drug_pairs,
           _want_time=False):
    in_maps = prep_inputs(h, params, row, col, head_ids, tail_ids, drug_pairs)
    if "nc" not in _PROGRAM_CACHE:
        _PROGRAM_CACHE["nc"] = build_program()
    nc = _PROGRAM_CACHE["nc"]
    res = run_bass_kernel_spmd(nc, in_maps, list(range(NCORES)),
                               trace=_want_time)
    g_out = np.concatenate([res.results[c]["goutT"].T for c in range(NCORES)], axis=0)
    closs = np.float32(sum(float(res.results[c]["closs"][0, 0])
                           for c in range(NCORES)) / (2 * B))
    if _want_time:
        return (g_out.astype(np.float32), closs), res
    return g_out.astype(np.float32), closs


# revision 39
# speedup vs baseline: 1.2347x; 1.1185x over previous
"""Trainium2 Bass kernel for nn_BKG_encoder (sparse graph-transformer encoder).

Strategy:
- Pure data parallelism: 16 of the 128 independent 64-node subgraphs per core.
- Sparse attention (fixed 16 out-edges per node, edges stay inside each
  64-node graph) is recomputed as dense 64x64 masked attention, where the
  mask is the edge multiplicity matrix (built host-side from row/col).
  exp(S - C) * mult, row-normalized, is mathematically identical to the
  reference's edge softmax.
- Activations are stored channel-on-partition ("transposed", hT = (256,
  1024-local-nodes)) so projections/FFN/BN are natural; attention S^T is
  computed per head with 64x64 graph blocks packed 2-per-128-partitions.
- BatchNorm is exact: per-core sums go through a tiny (128,4) AllReduce.
- fp16 is used only inside attention (q/k/v/exp/attn); everything dense
  runs fp32r on the PE (full speed at free-dim >= 256) with fp32 storage.
"""
import os
import sys

for _p in ("/opt/trn_rl_repo", "/opt/pypackages"):
    if _p not in sys.path:
        sys.path.insert(0, _p)

import numpy as np

import concourse.bass as bass
import concourse.bacc as bacc
import concourse.tile as tile
from concourse import mybir
from concourse.masks import make_identity
from concourse.bass_utils import run_bass_kernel_spmd

F32 = mybir.dt.float32
F32R = mybir.dt.float32r
F16 = mybir.dt.float16

HID = 256
HEADS = 8
HD = HID // HEADS          # 32
LAYERS = 3
N = 8192
B = 128
NPER = N // B              # 64
DEG = 16
EPS = 1e-5
SCALE = HD ** -0.5

NCORES = 8
BC = B // NCORES           # 16 graphs per core
NL = N // NCORES           # 1024 nodes per core
PAIRS = NL // 128          # 8 pairs of graphs (128-node blocks)
EXPC = 6.0                 # constant subtracted inside exp for fp16 range

# head-major channel permutation: PERM[32h + d] = d*HEADS + h
PERM = np.array([d * HEADS + h for h in range(HEADS) for d in range(HD)], np.int64)

AL = mybir.AluOpType


def r32(ap):
    return ap.bitcast(F32R)


def _ap_append(ap, stride, size):
    """Append an innermost free dim [stride, size] to an AP."""
    return bass.AP(tensor=ap.tensor, offset=ap.offset, ap=[*ap.ap, [stride, size]])


def _ap_insert0(ap, size):
    """Insert a stride-0 dim after the partition dim (broadcast copies)."""
    return bass.AP(tensor=ap.tensor, offset=ap.offset,
                   ap=[ap.ap[0], [0, size], *ap.ap[1:]])


def build_program():
    nc = bacc.Bacc("TRN2", target_bir_lowering=False, debug=False,
                   num_devices=NCORES)

    # ---- DRAM parameters (per-core inputs) ----
    d_hT = nc.declare_dram_parameter("hT", [HID, NL], F32, isOutput=False)
    d_mask = nc.declare_dram_parameter("maskT", [128, 512], F16, isOutput=False)
    d_wq = nc.declare_dram_parameter("wq", [LAYERS, HID, HID], F16, isOutput=False)
    d_wk = nc.declare_dram_parameter("wk", [LAYERS, HID, HID], F16, isOutput=False)
    d_wv = nc.declare_dram_parameter("wv", [LAYERS, HID, HID], F16, isOutput=False)
    d_wo = nc.declare_dram_parameter("wo", [LAYERS, HID, HID], F16, isOutput=False)
    d_w1 = nc.declare_dram_parameter("w1", [LAYERS, HID, 2 * HID], F16, isOutput=False)
    d_w2 = nc.declare_dram_parameter("w2", [LAYERS, 2 * HID, HID], F16, isOutput=False)
    d_bq = nc.declare_dram_parameter("bq", [LAYERS, HID, 1], F32, isOutput=False)
    d_bk = nc.declare_dram_parameter("bk", [LAYERS, HID, 1], F32, isOutput=False)
    d_bv = nc.declare_dram_parameter("bv", [LAYERS, 1, HID], F16, isOutput=False)
    d_b1 = nc.declare_dram_parameter("b1", [LAYERS, 2 * HID, 1], F32, isOutput=False)
    d_g1 = nc.declare_dram_parameter("g1", [LAYERS, HID, 1], F32, isOutput=False)
    d_be1 = nc.declare_dram_parameter("be1", [LAYERS, HID, 1], F32, isOutput=False)
    d_g2 = nc.declare_dram_parameter("g2", [LAYERS, HID, 1], F32, isOutput=False)
    d_be2 = nc.declare_dram_parameter("be2", [LAYERS, HID, 1], F32, isOutput=False)
    d_wh = nc.declare_dram_parameter("wh", [HID, HID], F16, isOutput=False)
    d_wb = nc.declare_dram_parameter("wb", [2 * HID, HID], F16, isOutput=False)
    d_cen = nc.declare_dram_parameter("censel", [HID, 2 * BC], F32, isOutput=False)

    d_gout = nc.declare_dram_parameter("goutT", [HID, BC], F32, isOutput=True)
    d_closs = nc.declare_dram_parameter("closs", [1, 1], F32, isOutput=True)

    with tile.TileContext(nc) as tc:
        with (
            tc.tile_pool(name="consts", bufs=1) as cp,
            tc.tile_pool(name="acts", bufs=2) as ap_,
            tc.tile_pool(name="qkp", bufs=2) as qkp,
            tc.tile_pool(name="vep", bufs=9) as vep,
            tc.tile_pool(name="eap", bufs=5) as eap,
            tc.tile_pool(name="etp", bufs=2) as etp,
            tc.tile_pool(name="a16p", bufs=2) as a16p,
            tc.tile_pool(name="atp2", bufs=2) as atp2,
            tc.tile_pool(name="ffnp", bufs=2) as fp_,
            tc.tile_pool(name="small", bufs=2) as sp_,
            tc.tile_pool(name="ps", bufs=4, space="PSUM") as pps,
            tc.tile_pool(name="pbig", bufs=2, space="PSUM") as ppb,
            tc.tile_pool(name="dram", bufs=4, space="DRAM") as dp_,
        ):
            sdma = nc.sync.dma_start

            # warmup collective: absorbs ncfw first-collective setup (~20us)
            # while the weight DMAs stream in
            warm_sb = sp_.tile([1, 1], F32, tag="warm")
            nc.vector.memset(warm_sb[:], 0.0)
            win = dp_.tile([1, 1], F32, tag="warm_in")
            wout = dp_.tile([1, 1], F32, tag="warm_out")
            nc.gpsimd.dma_start(out=win[:], in_=warm_sb[:])
            nc.gpsimd.collective_compute(
                "AllReduce", AL.add, replica_groups=[list(range(NCORES))],
                ins=[win.opt()], outs=[wout.opt()])

            # ---- load constants ----
            id16 = cp.tile([128, 128], F16, tag="id16")
            make_identity(nc, id16[:])

            mask_sb = cp.tile([128, 512], F16, tag="mask")
            sdma(out=mask_sb[:], in_=d_mask[:])
            mask2_sb = cp.tile([128, 1024], F16, tag="mask2")
            nc.vector.tensor_copy(out=mask2_sb[:].rearrange("p (a x) -> p a x", a=2),
                                  in_=_ap_insert0(mask_sb[:], 2))

            ones_row = cp.tile([1, 128], F16, tag="ones_row")   # k=1 lhsT, value 1
            nc.vector.memset(ones_row[:], 1.0)
            ones_inv64 = cp.tile([1, 128], F32, tag="ones_i64")  # value 1/64
            nc.vector.memset(ones_inv64[:], 1.0 / NPER)
            ones_col = cp.tile([128, 1], F32, tag="ones_col")    # closs rhs
            nc.vector.memset(ones_col[:], 1.0)
            magic = cp.tile([128, 1], mybir.dt.int32, tag="magic")
            nc.vector.memset(magic[:], 0x5F3759DF)
            negC = cp.tile([128, 1], F32, tag="negC")
            nc.vector.memset(negC[:], -EXPC)
            zbias = cp.tile([128, 1], F32, tag="zbias")
            nc.vector.memset(zbias[:], 0.0)

            # ---- batched weight loads: one DMA per class, spread over
            # ---- multiple engines' DMA queues ----
            _dmaeng = [nc.sync, nc.scalar, nc.gpsimd]
            _ei = [0]

            def bdma(out, in_):
                e = _dmaeng[_ei[0] % len(_dmaeng)]
                _ei[0] += 1
                e.dma_start(out=out, in_=in_)

            def load_mat(dram, nchunk, cols, dtype, tag):
                """dram (LAYERS, nchunk*128, cols) -> sbuf (128, L*nchunk*cols),
                one DMA. Returns [L][chunk] -> (128, cols) view list."""
                t = cp.tile([128, LAYERS * nchunk * cols], dtype, tag=tag, name=tag)
                bdma(t[:].rearrange("p (l k c) -> p l k c", l=LAYERS, k=nchunk),
                     dram.rearrange("l (k p) c -> p l k c", p=128))
                return [[t[:, (L * nchunk + kc) * cols:(L * nchunk + kc + 1) * cols]
                         for kc in range(nchunk)] for L in range(LAYERS)]

            def load_vecs(dram, nchunk, tag):
                """dram (LAYERS, nchunk*128, 1) -> [L][cc] -> (128, 1) views."""
                t = cp.tile([128, LAYERS * nchunk], F32, tag=tag, name=tag)
                bdma(t[:].rearrange("p (l k) -> p l k", l=LAYERS),
                     dram.rearrange("l (k p) c -> p l (k c)", p=128))
                return [[t[:, L * nchunk + cc:L * nchunk + cc + 1]
                         for cc in range(nchunk)] for L in range(LAYERS)]

            wq_sb = load_mat(d_wq, 2, HID, F16, "wq_all")
            wk_sb = load_mat(d_wk, 2, HID, F16, "wk_all")
            wv_sb = load_mat(d_wv, 2, HID, F16, "wv_all")
            wo_sb = load_mat(d_wo, 2, HID, F16, "wo_all")
            w1_sb = load_mat(d_w1, 2, 2 * HID, F16, "w1_all")
            w2_sb = load_mat(d_w2, 4, HID, F16, "w2_all")
            bq_sb = load_vecs(d_bq, 2, "bq_all")
            bk_sb = load_vecs(d_bk, 2, "bk_all")
            b1_sb = load_vecs(d_b1, 4, "b1_all")
            g1_sb = load_vecs(d_g1, 2, "g1_all")
            be1_sb = load_vecs(d_be1, 2, "be1_all")
            g2_sb = load_vecs(d_g2, 2, "g2_all")
            be2_sb = load_vecs(d_be2, 2, "be2_all")

            bv_all = cp.tile([1, LAYERS * HID], F16, tag="bv_all")
            bdma(bv_all[:].rearrange("p (l c) -> p l c", l=LAYERS),
                 d_bv.rearrange("l p c -> p l c"))
            bv_sb = [bv_all[:, L * HID:(L + 1) * HID] for L in range(LAYERS)]

            wh_all = cp.tile([128, 2 * HID], F16, tag="wh_all")
            bdma(wh_all[:].rearrange("p (k c) -> p k c", k=2),
                 d_wh.rearrange("(k p) c -> p k c", p=128))
            wh_sb = [wh_all[:, kc * HID:(kc + 1) * HID] for kc in range(2)]

            wb_all = cp.tile([128, 4 * HID], F16, tag="wb_all")
            bdma(wb_all[:].rearrange("p (k c) -> p k c", k=4),
                 d_wb.rearrange("(k p) c -> p k c", p=128))
            wb_sb = [wb_all[:, kb * HID:(kb + 1) * HID] for kb in range(4)]

            cen_all = cp.tile([128, 4 * BC], F32, tag="cen_all")
            bdma(cen_all[:].rearrange("p (k c) -> p k c", k=2),
                 d_cen.rearrange("(k p) c -> p k c", p=128))
            cen_sb = [cen_all[:, cc * 2 * BC:(cc + 1) * 2 * BC] for cc in range(2)]

            # ---- persistent Vext tiles: pad layout (zeros + ones columns)
            # is initialized once; only the V data is rewritten per layer ----
            vext = []
            for p in range(PAIRS):
                vx = cp.tile([128, HEADS * 66], F16, tag=f"vext{p}", name=f"vext{p}")
                nc.vector.memset(vx[:], 0.0)
                vxp = vx[:].ap[0][0]
                vxoff = vx[:].offset
                for half in range(2):
                    off = vxoff + half * (64 * vxp + 33)
                    nc.vector.memset(
                        bass.AP(tensor=vx[:].tensor, offset=off + 32,
                                ap=[[vxp, 64], [66, 8]]), 1.0)
                vext.append(vx)

            # ---- initial activations (f32 + fp16 shadow for PE operands) ----
            hcur, h16 = [], []
            for cc in range(2):
                t = ap_.tile([128, NL], F32, tag=f"hc{cc}")
                sdma(out=t[:], in_=d_hT[cc * 128:(cc + 1) * 128, :])
                hcur.append(t)
                t16 = qkp.tile([128, NL], F16, tag=f"h16{cc}", name=f"h16{cc}")
                nc.any.tensor_copy(out=t16[:], in_=t[:])
                h16.append(t16)

            # ---- helpers ----
            def rsqrt_newton(out, x, tmpname):
                """out = 1/sqrt(x), x (128,1) f32, via bit-trick + 2 Newton."""
                xi = x.bitcast(mybir.dt.int32)
                t1 = sp_.tile([128, 1], mybir.dt.int32, tag=tmpname + "i")
                nc.vector.tensor_scalar(out=t1[:], in0=xi, scalar1=1, scalar2=None,
                                        op0=AL.arith_shift_right)
                yi = sp_.tile([128, 1], mybir.dt.int32, tag=tmpname + "y")
                nc.vector.tensor_tensor(out=yi[:], in0=magic[:], in1=t1[:], op=AL.subtract)
                y = yi.bitcast(F32)
                xh = sp_.tile([128, 1], F32, tag=tmpname + "xh")
                nc.vector.tensor_scalar(out=xh[:], in0=x, scalar1=0.5, scalar2=None,
                                        op0=AL.mult)
                for it in range(2):
                    y2 = sp_.tile([128, 1], F32, tag=tmpname + "y2")
                    nc.vector.tensor_tensor(out=y2[:], in0=y, in1=y, op=AL.mult)
                    t2 = sp_.tile([128, 1], F32, tag=tmpname + "t2")
                    nc.vector.tensor_tensor(out=t2[:], in0=y2[:], in1=xh[:], op=AL.mult)
                    u = sp_.tile([128, 1], F32, tag=tmpname + "u")
                    nc.vector.tensor_scalar(out=u[:], in0=t2[:], scalar1=1.5, scalar2=-1.0,
                                            op0=AL.subtract, op1=AL.mult)
                    yn = sp_.tile([128, 1], F32, tag=tmpname + "yn" + str(it))
                    nc.vector.tensor_tensor(out=yn[:], in0=u[:], in1=y, op=AL.mult)
                    y = yn[:]
                nc.vector.tensor_copy(out=out, in_=y)

            def batchnorm(pre, gam, bet, bnname):
                """Global BN over all N nodes. pre: [2 x (128, NL) f32 tiles].
                Returns (scale, bias) per-chunk (128,1) f32 tiles."""
                arin = sp_.tile([128, 4], F32, tag=bnname + "in")
                for cc in range(2):
                    xg = pre[cc][:].rearrange("p (s f) -> p s f", f=512)
                    st = sp_.tile([128, 2, 6], F32, tag=bnname + "st")
                    for s in range(2):
                        nc.vector.bn_stats(out=st[:, s, :], in_=xg[:, s, :])
                    mv = sp_.tile([128, 2], F32, tag=bnname + "mv")
                    nc.vector.bn_aggr(out=mv[:], in_=st[:])
                    # arin[:, 2cc] = mean ; arin[:, 2cc+1] = var + mean^2
                    nc.vector.tensor_copy(out=arin[:, 2 * cc:2 * cc + 1], in_=mv[:, 0:1])
                    m2 = sp_.tile([128, 1], F32, tag=bnname + "m2")
                    nc.vector.tensor_tensor(out=m2[:], in0=mv[:, 0:1], in1=mv[:, 0:1],
                                            op=AL.mult)
                    nc.vector.tensor_tensor(out=arin[:, 2 * cc + 1:2 * cc + 2],
                                            in0=mv[:, 1:2], in1=m2[:], op=AL.add)
                bin_ = dp_.tile([128, 4], F32, tag=bnname + "bi")
                bout = dp_.tile([128, 4], F32, tag=bnname + "bo")
                sdma(out=bin_[:], in_=arin[:])
                nc.gpsimd.collective_compute(
                    "AllReduce", AL.add,
                    replica_groups=[list(range(NCORES))],
                    ins=[bin_.opt()], outs=[bout.opt()],
                )
                arout = sp_.tile([128, 4], F32, tag=bnname + "out")
                sdma(out=arout[:], in_=bout[:])
                scales, biases = [], []
                for cc in range(2):
                    gm = sp_.tile([128, 1], F32, tag=bnname + f"gm{cc}")
                    nc.vector.tensor_scalar(out=gm[:], in0=arout[:, 2 * cc:2 * cc + 1],
                                            scalar1=1.0 / NCORES, scalar2=None, op0=AL.mult)
                    gv = sp_.tile([128, 1], F32, tag=bnname + f"gv{cc}")
                    nc.vector.tensor_scalar(out=gv[:], in0=arout[:, 2 * cc + 1:2 * cc + 2],
                                            scalar1=1.0 / NCORES, scalar2=EPS,
                                            op0=AL.mult, op1=AL.add)
                    gm2 = sp_.tile([128, 1], F32, tag=bnname + f"gm2{cc}")
                    nc.vector.tensor_tensor(out=gm2[:], in0=gm[:], in1=gm[:], op=AL.mult)
                    nc.vector.tensor_tensor(out=gv[:], in0=gv[:], in1=gm2[:], op=AL.subtract)
                    rstd = sp_.tile([128, 1], F32, tag=bnname + f"rs{cc}")
                    rsqrt_newton(rstd[:], gv[:], bnname + f"nw{cc}")
                    sc = sp_.tile([128, 1], F32, tag=bnname + f"sc{cc}")
                    nc.vector.tensor_tensor(out=sc[:], in0=gam[cc][:], in1=rstd[:], op=AL.mult)
                    t = sp_.tile([128, 1], F32, tag=bnname + f"t{cc}")
                    nc.vector.tensor_tensor(out=t[:], in0=gm[:], in1=sc[:], op=AL.mult)
                    bi = sp_.tile([128, 1], F32, tag=bnname + f"bi{cc}")
                    nc.vector.tensor_tensor(out=bi[:], in0=bet[cc][:], in1=t[:], op=AL.subtract)
                    scales.append(sc)
                    biases.append(bi)
                return scales, biases

            # ================= layers =================
            for L in range(LAYERS):
                # ---- Q, K projections (transposed out, fp16) ----
                q16, k16 = [], []
                for (w_sb, b_sb, dst) in ((wq_sb[L], bq_sb[L], q16),
                                          (wk_sb[L], bk_sb[L], k16)):
                    for cc in range(2):
                        pq = ppb.tile([128, 1024], F32, tag="pbig")
                        for nh in range(2):
                            for kc in range(2):
                                nc.tensor.matmul(
                                    out=pq[:, nh * 512:(nh + 1) * 512],
                                    lhsT=w_sb[kc][:, cc * 128:(cc + 1) * 128],
                                    rhs=h16[kc][:, nh * 512:(nh + 1) * 512],
                                    start=(kc == 0), stop=(kc == 1))
                        t = qkp.tile([128, NL], F16, tag=f"qk{len(dst)}")
                        nc.scalar.activation(out=t[:], in_=pq[:],
                                             func=mybir.ActivationFunctionType.Identity,
                                             bias=b_sb[cc][:], scale=1.0)
                        dst.append(t)

                # ---- V projection (node-major) + Vext2: per head h, cols
                # [66h:66h+33) hold [V_h|1] valid on rows 0:64 (graph 2p) and
                # [66h+33:66h+66) valid on rows 64:128 (graph 2p+1); zeros
                # elsewhere. This lets one full-K (128) AV matmul produce both
                # graphs separated by column. ----
                for p in range(PAIRS):
                    pv = pps.tile([128, 512], F32, tag="ps")
                    for kc in range(2):
                        nc.tensor.matmul(
                            out=pv[:, 0:256],
                            lhsT=h16[kc][:, p * 128:(p + 1) * 128],
                            rhs=wv_sb[L][kc][:],
                            start=(kc == 0), stop=False)
                    nc.tensor.matmul(
                        out=pv[:, 0:256],
                        lhsT=ones_row[:],
                        rhs=bv_sb[L][:],
                        start=False, stop=True)
                    vx = vext[p]
                    vxp = vx[:].ap[0][0]
                    vxoff = vx[:].offset
                    for half in range(2):
                        off = vxoff + half * (64 * vxp + 33)
                        nc.vector.tensor_copy(
                            out=bass.AP(tensor=vx[:].tensor, offset=off,
                                        ap=[[vxp, 64], [66, 8], [1, 32]]),
                            in_=pv[64 * half:64 * half + 64, 0:256]
                                .rearrange("p (h x) -> p h x", x=32))

                # ---- S^T: 2 heads per psum tile; batched exp + mask ----
                ea16 = []
                for hp in range(HEADS // 2):
                    psx = ppb.tile([128, 1024], F32, tag="pbig")
                    for hh in range(2):
                        h = 2 * hp + hh
                        hc, hr = h // 4, 32 * (h % 4)
                        for g in range(BC):
                            half, blk = g % 2, g // 2
                            nc.tensor.matmul(
                                out=psx[64 * half:64 * half + 64,
                                        512 * hh + 64 * blk:512 * hh + 64 * blk + 64],
                                lhsT=k16[hc][hr:hr + 32, 64 * g:64 * g + 64],
                                rhs=q16[hc][hr:hr + 32, 64 * g:64 * g + 64],
                                start=True, stop=True,
                                tile_position=(hr, 64 * half))
                    et = etp.tile([128, 1024], F16, tag="et")
                    nc.scalar.activation(out=et[:], in_=psx[:],
                                         func=mybir.ActivationFunctionType.Exp,
                                         bias=negC[:], scale=1.0)
                    ea = eap.tile([128, 1024], F16, tag="ea")
                    nc.vector.tensor_tensor(out=ea[:], in0=et[:], in1=mask2_sb[:],
                                            op=AL.mult)
                    ea16.append(ea)

                # ---- AV: one full-K matmul per (head, pair); normalize,
                # ---- transpose back to channel-major ----
                attnT = [atp2.tile([128, NL], F16, tag=f"attnT{cc}", name=f"attnT{cc}")
                         for cc in range(2)]
                for p in range(PAIRS):
                    pav = ppb.tile([128, 1024], F32, tag="pbig")
                    ppitch = pav[:].ap[0][0]
                    poff = pav[:].offset
                    for h in range(HEADS):
                        hcol = 512 * (h // 4) + 66 * (h % 4)
                        nc.tensor.matmul(
                            out=pav[0:64, hcol:hcol + 66],
                            lhsT=ea16[h // 2][:, 512 * (h % 2) + 64 * p:
                                              512 * (h % 2) + 64 * p + 64],
                            rhs=vext[p][:, 66 * h:66 * h + 66],
                            start=True, stop=True)
                    rpi = sp_.tile([64, 16], F32, tag="rpi")
                    nc.vector.reciprocal(
                        out=rpi[:].rearrange("p (g m x) -> p g m x", g=2, m=4),
                        in_=bass.AP(tensor=pav[:].tensor, offset=poff + 32,
                                    ap=[[ppitch, 64], [512, 2], [66, 4], [33, 2]]))
                    a16 = a16p.tile([128, 256], F16, tag="a16")
                    for half in range(2):
                        rh = rpi[:].rearrange("p (g m x) -> p g m x", g=2, m=4)[:, :, :, half]
                        nc.vector.tensor_tensor(
                            out=a16[64 * half:64 * half + 64, :]
                                .rearrange("p (g m c) -> p g m c", g=2, m=4),
                            in0=bass.AP(tensor=pav[:].tensor,
                                        offset=poff + 33 * half,
                                        ap=[[ppitch, 64], [512, 2], [66, 4], [1, 32]]),
                            in1=_ap_append(rh, 0, 32), op=AL.mult)
                    pt = pps.tile([128, 528], F16, tag="ps", name="pt")
                    for cc in range(2):
                        nc.tensor.transpose(out=pt[:, 128 * cc:128 * cc + 128],
                                            in_=a16[:, 128 * cc:128 * cc + 128],
                                            identity=id16[:])
                        nc.any.tensor_copy(out=attnT[cc][:, 128 * p:128 * p + 128],
                                           in_=pt[:, 128 * cc:128 * cc + 128])

                # ---- O projection + residual ----
                hres = []
                for cc in range(2):
                    po = ppb.tile([128, 1024], F32, tag="pbig")
                    for nh in range(2):
                        for kc in range(2):
                            nc.tensor.matmul(
                                out=po[:, nh * 512:(nh + 1) * 512],
                                lhsT=wo_sb[L][kc][:, cc * 128:(cc + 1) * 128],
                                rhs=attnT[kc][:, nh * 512:(nh + 1) * 512],
                                start=(kc == 0), stop=(kc == 1))
                    hr_ = ap_.tile([128, NL], F32, tag=f"res{cc}")
                    nc.vector.tensor_tensor(out=hr_[:], in0=po[:], in1=hcur[cc][:], op=AL.add)
                    hres.append(hr_)

                # ---- BN1 + GELU ----
                sc1, bi1 = batchnorm(hres, g1_sb[L], be1_sb[L], f"bn1_{L}")
                x2, x216 = [], []
                for cc in range(2):
                    t = ap_.tile([128, NL], F32, tag=f"x2{cc}")
                    nc.scalar.activation(out=t[:], in_=hres[cc][:],
                                         func=mybir.ActivationFunctionType.Gelu,
                                         bias=bi1[cc][:], scale=sc1[cc][:])
                    x2.append(t)
                    t16 = qkp.tile([128, NL], F16, tag=f"x216{cc}", name=f"x216{cc}")
                    nc.any.tensor_copy(out=t16[:], in_=t[:])
                    x216.append(t16)

                # ---- FFN ----
                ffn = []
                for m4 in range(4):
                    pf = ppb.tile([128, 1024], F32, tag="pbig")
                    for nh in range(2):
                        for kc in range(2):
                            nc.tensor.matmul(
                                out=pf[:, nh * 512:(nh + 1) * 512],
                                lhsT=w1_sb[L][kc][:, m4 * 128:(m4 + 1) * 128],
                                rhs=x216[kc][:, nh * 512:(nh + 1) * 512],
                                start=(kc == 0), stop=(kc == 1))
                    t = fp_.tile([128, NL], F16, tag=f"ffn{m4}")
                    nc.scalar.activation(out=t[:], in_=pf[:],
                                         func=mybir.ActivationFunctionType.Gelu,
                                         bias=b1_sb[L][m4][:], scale=1.0)
                    ffn.append(t)

                pre2 = []
                for cc in range(2):
                    pf2 = ppb.tile([128, 1024], F32, tag="pbig")
                    for nh in range(2):
                        for kc4 in range(4):
                            nc.tensor.matmul(
                                out=pf2[:, nh * 512:(nh + 1) * 512],
                                lhsT=w2_sb[L][kc4][:, cc * 128:(cc + 1) * 128],
                                rhs=ffn[kc4][:, nh * 512:(nh + 1) * 512],
                                start=(kc4 == 0), stop=(kc4 == 3))
                    t = ap_.tile([128, NL], F32, tag=f"res{cc}")
                    nc.vector.tensor_tensor(out=t[:], in0=pf2[:], in1=x2[cc][:], op=AL.add)
                    pre2.append(t)

                # ---- BN2 ----
                sc2, bi2 = batchnorm(pre2, g2_sb[L], be2_sb[L], f"bn2_{L}")
                hnew, h16new = [], []
                for cc in range(2):
                    t = ap_.tile([128, NL], F32, tag=f"hc{cc}")
                    nc.vector.tensor_scalar(out=t[:], in0=pre2[cc][:], scalar1=sc2[cc][:],
                                            scalar2=bi2[cc][:], op0=AL.mult, op1=AL.add)
                    hnew.append(t)
                    t16 = qkp.tile([128, NL], F16, tag=f"h16{cc}", name=f"h16n{cc}")
                    nc.scalar.activation(out=t16[:], in_=pre2[cc][:],
                                         func=mybir.ActivationFunctionType.Identity,
                                         bias=bi2[cc][:], scale=sc2[cc][:])
                    h16new.append(t16)
                hcur, h16 = hnew, h16new

            # ================= readout =================
            # hw = h @ W_h (channel-major), cast fp16
            hw16 = []
            for cc in range(2):
                ph = ppb.tile([128, 1024], F32, tag="pbig")
                for nh in range(2):
                    for kc in range(2):
                        nc.tensor.matmul(
                            out=ph[:, nh * 512:(nh + 1) * 512],
                            lhsT=wh_sb[kc][:, cc * 128:(cc + 1) * 128],
                            rhs=h16[kc][:, nh * 512:(nh + 1) * 512],
                            start=(kc == 0), stop=(kc == 1))
                t = sp_.tile([128, NL], F16, tag=f"hw16{cc}")
                nc.any.tensor_copy(out=t[:], in_=ph[:])
                hw16.append(t)

            # base vectors: columns head (v=0) and tail (v=1) of each graph
            bv4 = []
            for part, v in ((0, 0), (1, 0), (0, 1), (1, 1)):
                # order: [h_head chunk0, h_head chunk1, h_tail chunk0, h_tail chunk1]
                t = sp_.tile([128, BC], F16, tag=f"bv4_{len(bv4)}")
                hv = hcur[part][:].rearrange("p (g v) -> p g v", v=NPER)
                nc.vector.tensor_copy(out=t[:], in_=hv[:, :, v])
                bv4.append(t)

            tb16 = []
            for m in range(2):
                pt = pps.tile([128, 512], F32, tag="ps")
                for kb in range(4):
                    nc.tensor.matmul(
                        out=pt[:, 0:BC],
                        lhsT=wb_sb[kb][:, m * 128:(m + 1) * 128],
                        rhs=bv4[kb][:],
                        start=(kb == 0), stop=(kb == 3))
                t = sp_.tile([128, BC], F16, tag=f"tb16{m}")
                nc.any.tensor_copy(out=t[:], in_=pt[:, 0:BC])
                tb16.append(t)

            # per-node scores s[g, n] via per-graph (1 x 64) matmuls
            ps4 = pps.tile([128, 512], F32, tag="ps")
            for g in range(BC):
                q, blk = g // 4, g % 4
                for cc in range(2):
                    # lhsT column replicated 32x (stride-0) so the matmul
                    # fills a whole 32-row group: no unwritten-psum reads.
                    nc.tensor.matmul(
                        out=ps4[32 * q:32 * q + 32, 64 * blk:64 * blk + 64],
                        lhsT=_ap_append(tb16[cc][:, g:g + 1], 0, 32),
                        rhs=hw16[cc][:, 64 * g:64 * g + 64],
                        start=(cc == 0), stop=(cc == 1),
                        tile_position=(0, 32 * q))
            # softmax in the sparse (rows 0/32/64/96) layout; DVE cost only
            # depends on the free size, and the garbage rows are never read.
            s_sb = sp_.tile([128, 256], F32, tag="s_sb")
            nc.vector.tensor_copy(out=s_sb[:], in_=ps4[:, 0:256])
            sv = s_sb[:].rearrange("p (b f) -> p b f", f=64)
            negm = sp_.tile([128, 4], F32, tag="negm")
            nc.vector.tensor_reduce(out=negm[:], in_=sv, axis=mybir.AxisListType.X,
                                    op=AL.max, negate=True)
            e_sb = sp_.tile([128, 256], F32, tag="e_sb")
            ev = e_sb[:].rearrange("p (b f) -> p b f", f=64)
            nc.vector.tensor_tensor(out=ev, in0=sv, in1=_ap_append(negm[:], 0, 64),
                                    op=AL.add)
            nc.scalar.activation(out=e_sb[:], in_=e_sb[:],
                                 func=mybir.ActivationFunctionType.Exp,
                                 bias=zbias[:], scale=1.0)
            se = sp_.tile([128, 4], F32, tag="se")
            nc.vector.tensor_reduce(out=se[:], in_=ev, axis=mybir.AxisListType.X,
                                    op=AL.add)
            ri = sp_.tile([128, 4], F32, tag="ri")
            nc.vector.reciprocal(out=ri[:], in_=se[:])
            a_sb = sp_.tile([128, 256], F32, tag="a_sb")
            nc.vector.tensor_tensor(out=a_sb[:].rearrange("p (b f) -> p b f", f=64),
                                    in0=ev, in1=_ap_append(ri[:], 0, 64), op=AL.mult)
            a_row = sp_.tile([1, NL], F32, tag="a_row")
            _pitch = a_sb[:].ap[0][0]
            a_gather = bass.AP(tensor=a_sb[:].tensor, offset=a_sb[:].offset,
                               ap=[[_pitch * 32, 4], [64, 4], [1, 64]])
            sdma(out=a_row[:], in_=a_gather)

            pab = ppb.tile([128, 1024], F32, tag="pbig")
            for nh in range(2):
                nc.tensor.matmul(out=pab[:, nh * 512:(nh + 1) * 512],
                                 lhsT=ones_inv64[:],
                                 rhs=a_row[:, nh * 512:(nh + 1) * 512],
                                 start=True, stop=True)
            for cc in range(2):
                pr = sp_.tile([128, NL], F32, tag=f"pr{cc}")
                nc.vector.tensor_tensor(out=pr[:], in0=hcur[cc][:], in1=pab[:], op=AL.mult)
                gsb = sp_.tile([128, BC], F32, tag=f"gsb{cc}")
                nc.vector.tensor_reduce(out=gsb[:], in_=pr[:].rearrange("p (g v) -> p g v", v=NPER),
                                        axis=mybir.AxisListType.X, op=AL.add)
                sdma(out=d_gout[cc * 128:(cc + 1) * 128, :], in_=gsb[:])

            # ---- center loss partial ----
            pcl = pps.tile([128, 512], F32, tag="ps")
            for cc in range(2):
                fh = sp_.tile([128, 2 * BC], F32, tag=f"fh{cc}")
                hv = hcur[cc][:].rearrange("p (g v) -> p g v", v=NPER)
                nc.vector.tensor_copy(out=fh[:, 0:BC], in_=hv[:, :, 0])
                nc.vector.tensor_copy(out=fh[:, BC:2 * BC], in_=hv[:, :, 1])
                d_ = sp_.tile([128, 2 * BC], F32, tag=f"d{cc}")
                nc.vector.tensor_tensor(out=d_[:], in0=fh[:], in1=cen_sb[cc][:],
                                        op=AL.subtract)
                dj = sp_.tile([128, 2 * BC], F32, tag=f"dj{cc}")
                dsum = sp_.tile([128, 1], F32, tag=f"dsum{cc}")
                nc.scalar.activation(out=dj[:], in_=d_[:],
                                     func=mybir.ActivationFunctionType.Square,
                                     bias=zbias[0:128, :], accum_out=dsum[:])
                nc.tensor.matmul(out=pcl[0:1, 0:1], lhsT=dsum[:], rhs=ones_col[:],
                                 start=(cc == 0), stop=(cc == 1))
            cl_sb = sp_.tile([1, 1], F32, tag="cl")
            nc.vector.tensor_copy(out=cl_sb[:], in_=pcl[0:1, 0:1])
            sdma(out=d_closs[:], in_=cl_sb[:])

    nc.compile()
    return nc


def prep_inputs(h, params, row, col, head_ids, tail_ids, drug_pairs):
    """Build the 8 per-core input maps."""
    h = np.asarray(h, np.float32)
    row = np.asarray(row); col = np.asarray(col)
    head_ids = np.asarray(head_ids); tail_ids = np.asarray(tail_ids)
    drug_pairs = np.asarray(drug_pairs)
    assert np.array_equal(head_ids, np.arange(B, dtype=head_ids.dtype) * NPER), \
        "unexpected head_ids layout"
    assert np.array_equal(tail_ids, head_ids + 1), "unexpected tail_ids layout"
    assert np.all(col // NPER == row // NPER), "edges cross graphs"

    # multiplicity mask
    M = np.zeros((B, NPER, NPER), np.float32)
    np.add.at(M, (row // NPER, row % NPER, col % NPER), 1.0)

    layers = params["layers"]
    wq = np.stack([np.asarray(p["q"][0], np.float32)[:, PERM] * np.float32(SCALE)
                   for p in layers]).astype(np.float16)
    wk = np.stack([np.asarray(p["k"][0], np.float32)[:, PERM] for p in layers]).astype(np.float16)
    wv = np.stack([np.asarray(p["v"][0], np.float32)[:, PERM] for p in layers]).astype(np.float16)
    wo = np.stack([np.asarray(p["o"][0], np.float32)[PERM, :] for p in layers]).astype(np.float16)
    w1 = np.stack([np.asarray(p["f1"][0], np.float32) for p in layers]).astype(np.float16)
    w2 = np.stack([np.asarray(p["f2"][0], np.float32) for p in layers]).astype(np.float16)
    bq = np.stack([np.asarray(p["q"][1], np.float32)[PERM] * np.float32(SCALE)
                   for p in layers]).reshape(LAYERS, HID, 1)
    bk = np.stack([np.asarray(p["k"][1], np.float32)[PERM] for p in layers]).reshape(LAYERS, HID, 1)
    bv = np.stack([np.asarray(p["v"][1], np.float32)[PERM] for p in layers]).reshape(LAYERS, 1, HID).astype(np.float16)
    b1 = np.stack([np.asarray(p["f1"][1], np.float32) for p in layers]).reshape(LAYERS, 2 * HID, 1)
    g1 = np.stack([np.asarray(p["bn1"][0], np.float32) for p in layers]).reshape(LAYERS, HID, 1)
    be1 = np.stack([np.asarray(p["bn1"][1], np.float32) for p in layers]).reshape(LAYERS, HID, 1)
    g2 = np.stack([np.asarray(p["bn2"][0], np.float32) for p in layers]).reshape(LAYERS, HID, 1)
    be2 = np.stack([np.asarray(p["bn2"][1], np.float32) for p in layers]).reshape(LAYERS, HID, 1)
    wh = np.asarray(params["W_h"], np.float32).astype(np.float16)
    wb = np.asarray(params["W_base"], np.float32).astype(np.float16)
    centers = np.asarray(params["centers"], np.float32)

    shared = dict(wq=wq, wk=wk, wv=wv, wo=wo, w1=w1, w2=w2, bq=bq, bk=bk, bv=bv,
                  b1=b1, g1=g1, be1=be1, g2=g2, be2=be2, wh=wh, wb=wb)
    shared = {k: np.ascontiguousarray(v) for k, v in shared.items()}

    in_maps = []
    for c in range(NCORES):
        hT = np.ascontiguousarray(h[c * NL:(c + 1) * NL].T)
        maskT = np.zeros((128, 512), np.float16)
        Mc = M[c * BC:(c + 1) * BC]
        for g in range(BC):
            half, blk = g % 2, g // 2
            maskT[64 * half:64 * half + 64, 64 * blk:64 * blk + 64] = Mc[g].T
        censel = np.zeros((HID, 2 * BC), np.float32)
        for g in range(BC):
            censel[:, g] = centers[drug_pairs[c * BC + g, 0]]
            censel[:, BC + g] = centers[drug_pairs[c * BC + g, 1]]
        m = dict(hT=hT, maskT=maskT, censel=np.ascontiguousarray(censel))
        m.update(shared)
        in_maps.append(m)
    return in_maps


_PROGRAM_CACHE = {}


def kernel(h, params, row, col, graph_id, head_ids, tail_ids, drug_pairs,
           _want_time=False):
    in_maps = prep_inputs(h, params, row, col, head_ids, tail_ids, drug_pairs)
    if "nc" not in _PROGRAM_CACHE:
        _PROGRAM_CACHE["nc"] = build_program()
    nc = _PROGRAM_CACHE["nc"]
    res = run_bass_kernel_spmd(nc, in_maps, list(range(NCORES)),
                               trace=_want_time)
    g_out = np.concatenate([res.results[c]["goutT"].T for c in range(NCORES)], axis=0)
    closs = np.float32(sum(float(res.results[c]["closs"][0, 0])
                           for c in range(NCORES)) / (2 * B))
    if _want_time:
        return (g_out.astype(np.float32), closs), res
    return g_out.astype(np.float32), closs
